# revision 1
# baseline (speedup 1.0000x reference)
"""Trainium2 Bass kernel for nn_DSSA (spiking self-attention block).

Sharding: data-parallel over B — core b handles batch element b (B=8, 8 cores).
All weights/constants are host-folded, cast to bf16 where safe, replicated.

Numerics design (validated against the jax reference in numpy, 8.8e-8 norm-rel):
- LIF spikes are encoded as {0, 2} in bf16 (exact), produced by one fused
  tensor_scalar (is_ge, mult-2) op; every consumer matmul's scale constants are
  pre-divided by 2 on the host (BN fold), and LIF thresholds are pre-scaled.
- lif(x) state kept in fp32 with the scaled recurrence U = 0.5*R + x
  (verified bit-identical spikes vs jax's v + (x-v)/2 order on the real data).
- attn / out LIF states and potentials in bf16 (threshold margins ~0.29).
- Patchify conv = 16 shifted K=384 matmuls with strided access patterns on the
  spike tile (no im2col materialization); conv output channels are permuted on
  the host so BN1's output lands directly in matmul-ready y1 / y2 layouts.
- BN1/BN2 folded into per-partition scale/bias (ACT engine, one pass).
- BN2 bias enters via a K=1 fp32r ones-matmul into the proj PSUM accumulation.
"""

import numpy as np
import ml_dtypes

import concourse.bacc as bacc
import concourse.mybir as mybir
from concourse.tile import TileContext
from concourse.bass_utils import run_bass_kernel_spmd

bf16np = ml_dtypes.bfloat16
F32 = mybir.dt.float32
F32R = mybir.dt.float32r
BF16 = mybir.dt.bfloat16
ALU = mybir.AluOpType
ACTF = mybir.ActivationFunctionType

T, B, C, H, W = 4, 8, 384, 32, 32
NH, CH, P = 12, 32, 4           # heads, head dim, patch
HP = H // P                      # 8
NP = HP * HP                     # 64 patches
N = H * W                        # 1024
CT = C // 128                    # 3 channel tiles
EPS = 1e-5

_CACHE = {}


def _build_program(repeat=1, hw_loop=0):
    nc = bacc.Bacc("TRN2", target_bir_lowering=False)

    x_in = nc.declare_dram_parameter("x", [T, CT, 128, N], F32, isOutput=False)
    wconv = nc.declare_dram_parameter("wconv", [6, 3, 128, 2048], BF16, isOutput=False)
    wproj = nc.declare_dram_parameter("wproj", [128, 9 * 128], BF16, isOutput=False)
    consts = nc.declare_dram_parameter("consts", [128, 24], F32, isOutput=False)
    aux = nc.declare_dram_parameter("aux", [1, 896], F32, isOutput=False)
    y_out = nc.declare_dram_parameter("y", [T, CT, 128, N], F32, isOutput=True)

    # consts columns: 0-5 A1p, 6-11 B1p, 12-17 gam1(pair), 18-20 gam2(ctile), 21-23 prA
    with TileContext(nc) as tc:
        with tc.tile_pool(name="sb", bufs=1) as sb:
            cst = sb.tile([128, 24], F32, tag="cst")
            nc.sync.dma_start(cst[:], consts[:])
            auxT = sb.tile([1, 896], F32R, tag="aux")
            nc.sync.dma_start(auxT[:], aux[:].bitcast(F32R))
            wpj = sb.tile([128, 9 * 128], BF16, tag="wpj")
            nc.sync.dma_start(wpj[:], wproj[:])

            # ---- persistent state / data tiles ----
            xt = [[sb.tile([128, N], F32, tag=f"x{t}{ct}", name=f"x{t}{ct}")
                   for ct in range(CT)] for t in range(T)]
            # x DMAs are interleaved with conv wave-1 weight DMAs below so the
            # PE can start accumulating as soon as sigma_x[ct=0] is ready.

            v = [sb.tile([128, N], F32, tag=f"v{ct}", name=f"v{ct}") for ct in range(CT)]
            sx = [sb.tile([128, T * N], BF16, tag=f"sx{ct}", name=f"sx{ct}") for ct in range(CT)]
            Gat = [sb.tile([128, N], BF16, tag=f"Gat{p}", name=f"Gat{p}") for p in range(6)]
            Got = [sb.tile([128, N], BF16, tag=f"Got{g}", name=f"Got{g}") for g in range(CT)]
            y1c = [sb.tile([128, T * NP], BF16, tag=f"y1c{g}", name=f"y1c{g}") for g in range(CT)]
            y2c = [sb.tile([128, T * NP], BF16, tag=f"y2c{g}", name=f"y2c{g}") for g in range(CT)]
            # block-diag lhsT tiles for MM2 (persistent, ping-pong by t parity;
            # off-diag zeroed once)
            L = [[sb.tile([128, 64], BF16, tag=f"L{s}{p}", name=f"L{s}{p}")
                  for p in range(6)] for s in range(2)]
            for s in range(2):
                for p in range(6):
                    nc.gpsimd.memset(L[s][p][:], 0.0)

            # ---- Phase B: x-LIF (scaled recurrence, fp32, spikes {0,2}) ----
            # ct-major so sigma_x[ct] completes early and conv wave 1 (kt-outer)
            # can start accumulating while later ct tiles are still in LIF.
            def emit_xlif(ct, xl):
                for t in range(T):
                    if t == 0:
                        U = xt[0][ct][:]
                    else:
                        Ut = xl.tile([128, N], F32, tag="xu", name=f"xu{ct}{t}")
                        nc.vector.scalar_tensor_tensor(
                            Ut[:], v[ct][:], 0.5, xt[t][ct][:], ALU.mult, ALU.add)
                        U = Ut[:]
                    nc.gpsimd.tensor_scalar(
                        sx[ct][:, t * N:(t + 1) * N], U, 2.0, 2.0, ALU.is_ge, ALU.mult)
                    if t < T - 1:
                        nc.vector.scalar_tensor_tensor(
                            v[ct][:], U, 2.0, U, ALU.is_lt, ALU.mult)

            # ---- Phases C+D: conv (16 shifted matmuls) + BN1, interleaved with
            #      the per-t attention chain. LIF states Gat/Got store the
            #      PRE-HALVED membrane (0.5*U*mask) so the update is a plain
            #      bf16 tensor_tensor add (DVE 2x mode); resets are TS(4x)+TT(2x).
            # free index of sx: t*1024 + 128*hp + 32*i + 4*wp + j
            sxr = [sx[ct].rearrange("c (t hp i wp j) -> c t hp i wp j",
                                    t=4, hp=8, i=4, wp=8, j=4) for ct in range(CT)]
            with tc.tile_pool(name="cw", bufs=4) as cw, \
                 tc.tile_pool(name="xl", bufs=2) as xl, \
                 tc.tile_pool(name="tl", bufs=2) as tl, \
                 tc.tile_pool(name="pm1p", bufs=2, space="PSUM") as pm1p:

                def emit_bn1(mt, pc):
                    dst = y1c[mt] if mt < 3 else y2c[mt - 3]
                    nc.scalar.activation(dst[:], pc[:], ACTF.Identity,
                                         bias=cst[:, 6 + mt:7 + mt],
                                         scale=cst[:, mt:mt + 1])

                def emit_conv_mt(cp, mt):
                    pc = cp.tile([128, T * NP], F32, tag="pcw2", name=f"pc{mt}")
                    for kt in range(3):
                        wt = cw.tile([128, 2048], BF16, tag="wc", name=f"wc{mt}{kt}")
                        nc.sync.dma_start(wt[:], wconv[mt, kt])
                        for ij in range(16):
                            i, j = ij // 4, ij % 4
                            nc.tensor.matmul(
                                pc[:], wt[:, ij * 128:(ij + 1) * 128],
                                sxr[kt][:, :, :, i, :, j],
                                start=(kt == 0 and ij == 0),
                                stop=(kt == 2 and ij == 15),
                                skip_group_check=True)
                    emit_bn1(mt, pc)

                def emit_ltrans(t):
                    # block-diag y2^T lhsT tiles (DVE 32x32 transposes)
                    Ls = L[t % 2]
                    for p in range(6):
                        g, jj = p // 2, p % 2
                        for bb in range(2):
                            c0 = t * 64 + 32 * bb
                            nc.vector.transpose(
                                Ls[p][32 * bb:32 * bb + 32, 0:32],
                                y2c[g][64 * jj:64 * jj + 32, c0:c0 + 32])
                            nc.vector.transpose(
                                Ls[p][64 + 32 * bb:96 + 32 * bb, 32:64],
                                y2c[g][64 * jj + 32:64 * jj + 64, c0:c0 + 32])

                sa = {}

                def emit_attn_pair(t, p):
                    g, jj = p // 2, p % 2
                    pm1 = pm1p.tile([128, N], F32, tag="pm1", name=f"pm1_{t}_{p}")
                    rA = 64 * jj
                    for nh in range(2):
                        nc.tensor.matmul(
                            pm1[0:64, nh * 512:(nh + 1) * 512],
                            y1c[g][rA:rA + 32, t * 64:(t + 1) * 64],
                            sx[g][rA:rA + 32, t * N + nh * 512:t * N + (nh + 1) * 512],
                            start=True, stop=True, tile_position=(rA, 0))
                        nc.tensor.matmul(
                            pm1[64:128, nh * 512:(nh + 1) * 512],
                            y1c[g][rA + 32:rA + 64, t * 64:(t + 1) * 64],
                            sx[g][rA + 32:rA + 64, t * N + nh * 512:t * N + (nh + 1) * 512],
                            start=True, stop=True, tile_position=(rA + 32, 64))
                    Pb = tl.tile([128, N], BF16, tag="Pb", name=f"Pb{t}{p}")
                    nc.scalar.copy(Pb[:], pm1[:])
                    if t == 0:
                        Ua = Pb[:]
                    else:
                        Uat = tl.tile([128, N], BF16, tag="Ua", name=f"Ua{t}{p}")
                        nc.vector.tensor_tensor(Uat[:], Gat[p][:], Pb[:], ALU.add)
                        Ua = Uat[:]
                    sat = tl.tile([128, N], BF16, tag=f"sa{p}", bufs=2, name=f"sa{t}{p}")
                    nc.gpsimd.tensor_scalar(
                        sat[:], Ua, cst[:, 12 + p:13 + p], 2.0, ALU.is_ge, ALU.mult)
                    sa[(t, p)] = sat
                    if t < T - 1:
                        m = tl.tile([128, N], BF16, tag="am", name=f"am{t}{p}")
                        nc.gpsimd.tensor_scalar(
                            m[:], Ua, cst[:, 12 + p:13 + p], 0.5, ALU.is_lt, ALU.mult)
                        nc.vector.tensor_tensor(Gat[p][:], Ua, m[:], ALU.mult)

                pools = {}
                so_by_t = {}

                def emit_mm2_outlif(t):
                    so = []
                    for g in range(CT):
                        po = pools["pop"].tile([128, N], F32, tag="po", name=f"po{t}{g}")
                        for jj in range(2):
                            p = 2 * g + jj
                            for nh in range(2):
                                nc.tensor.matmul(
                                    po[64 * jj:64 * jj + 64, nh * 512:(nh + 1) * 512],
                                    L[t % 2][p][:, 0:64],
                                    sa[(t, p)][:, nh * 512:(nh + 1) * 512],
                                    start=True, stop=True, tile_position=(0, 64 * jj))
                        Pb2 = tl.tile([128, N], BF16, tag="Pb2", name=f"Pb2_{t}{g}")
                        nc.scalar.copy(Pb2[:], po[:])
                        if t == 0:
                            Uo = Pb2[:]
                        else:
                            Uot = tl.tile([128, N], BF16, tag="Uo", name=f"Uo{t}{g}")
                            nc.vector.tensor_tensor(Uot[:], Got[g][:], Pb2[:], ALU.add)
                            Uo = Uot[:]
                        sot = tl.tile([128, N], BF16, tag=f"so{g}", bufs=1, name=f"so{t}{g}")
                        nc.gpsimd.tensor_scalar(
                            sot[:], Uo, cst[:, 18 + g:19 + g], 2.0, ALU.is_ge, ALU.mult)
                        so.append(sot)
                        if t < T - 1:
                            m = tl.tile([128, N], BF16, tag="om", name=f"om{t}{g}")
                            nc.gpsimd.tensor_scalar(
                                m[:], Uo, cst[:, 18 + g:19 + g], 0.5, ALU.is_lt, ALU.mult)
                            nc.vector.tensor_tensor(Got[g][:], Uo, m[:], ALU.mult)
                    so_by_t[t] = so

                def emit_proj_epi(t):
                    so = so_by_t[t]
                    for mt in range(CT):
                        for nh in range(2):
                            pj = pools["pjp"].tile([128, 512], F32, tag="pj", name=f"pj{t}{mt}{nh}")
                            for kt in range(3):
                                nc.tensor.matmul(
                                    pj[:], wpj[:, (mt * 3 + kt) * 128:(mt * 3 + kt + 1) * 128],
                                    so[kt][:, nh * 512:(nh + 1) * 512],
                                    start=(kt == 0), stop=False, skip_group_check=True)
                            nc.tensor.matmul(
                                pj[:], auxT[0:1, 512 + mt * 128:512 + (mt + 1) * 128],
                                auxT[0:1, 0:512],
                                start=False, stop=True, skip_group_check=True)
                            of = tl.tile([128, 512], F32, tag="of", name=f"of{t}{mt}{nh}")
                            nc.vector.scalar_tensor_tensor(
                                of[:], pj[:], cst[:, 21 + mt:22 + mt],
                                xt[t][mt][:, nh * 512:(nh + 1) * 512], ALU.mult, ALU.add)
                            nc.sync.dma_start(
                                y_out[t, mt, :, nh * 512:(nh + 1) * 512], of[:])

                # Schedule: x-LIF(ct) interleaved with conv wave 1 (kt-outer
                # over the y2 tiles, so PE accumulates on sigma_x[ct] while
                # LIF computes ct+1), then conv wave 2 (y1 tiles, mt-outer)
                # interleaved with the t=0 attention chain, then t=1..3 chains.
                # repeat>1 re-emits the whole body for slope-based HW timing;
                # hw_loop>0 wraps the body in an on-device For_i instead.
                from contextlib import ExitStack as _ES
                loop_ctx = _ES()
                if hw_loop:
                    loop_ctx.enter_context(tc.For_i(0, hw_loop, 1))
                for _rep in range(repeat):
                  with tc.tile_pool(name="cp1", bufs=1, space="PSUM") as cp1:
                      pcw1 = [cp1.tile([128, T * NP], F32, tag=f"pcw1_{m}",
                                       name=f"pcw1_{m}") for m in range(3)]
                      for t in range(T):
                          nc.sync.dma_start(xt[t][0][:], x_in[t, 0])
                      emit_xlif(0, xl)
                      for kt in range(3):
                          for mi, mt in enumerate((3, 4, 5)):
                              wt = cw.tile([128, 2048], BF16, tag="wc",
                                           name=f"w1c{mt}{kt}")
                              nc.sync.dma_start(wt[:], wconv[mt, kt])
                              if kt < 2 and mi == 0:
                                  # next ct's x tiles after this kt's weights
                                  for t in range(T):
                                      nc.sync.dma_start(xt[t][kt + 1][:],
                                                        x_in[t, kt + 1])
                              for ij in range(16):
                                  i, j = ij // 4, ij % 4
                                  nc.tensor.matmul(
                                      pcw1[mi][:], wt[:, ij * 128:(ij + 1) * 128],
                                      sxr[kt][:, :, :, i, :, j],
                                      start=(kt == 0 and ij == 0),
                                      stop=(kt == 2 and ij == 15),
                                      skip_group_check=True)
                          if kt < 2:
                              emit_xlif(kt + 1, xl)
                      for mi, mt in enumerate((3, 4, 5)):
                          emit_bn1(mt, pcw1[mi])
                      emit_ltrans(0)
                      with tc.tile_pool(name="cp2", bufs=1, space="PSUM") as cp2:
                          for g in range(CT):
                              emit_conv_mt(cp2, g)
                              emit_attn_pair(0, 2 * g)
                              emit_attn_pair(0, 2 * g + 1)
                  with tc.tile_pool(name="pop", bufs=1, space="PSUM") as pop_, \
                       tc.tile_pool(name="pjp", bufs=2, space="PSUM") as pjp_:
                      pools["pop"] = pop_
                      pools["pjp"] = pjp_
                      # software-pipelined: MM2/out-LIF of t, then MM1/attn of
                      # t+1 (critical path), then the deferred proj/epilogue of t
                      # deeper pipeline: MM1/attn of t+1 is emitted BEFORE
                      # MM2/out-LIF of t (L and sa are double-buffered), so
                      # the PE feeds the DVE chain a full stage ahead.
                      emit_ltrans(1)
                      for p in range(6):
                          emit_attn_pair(1, p)
                      emit_mm2_outlif(0)
                      for t in range(2, T):
                          emit_ltrans(t)
                          for p in range(6):
                              emit_attn_pair(t, p)
                          emit_proj_epi(t - 2)
                          emit_mm2_outlif(t - 1)
                      emit_proj_epi(T - 2)
                      emit_mm2_outlif(T - 1)
                      emit_proj_epi(T - 1)
                loop_ctx.close()
    nc.compile()
    return nc


def _host_prep(inputs):
    f32 = np.float32
    w_conv = inputs["w_conv"].astype(f32)
    w_proj = inputs["w_proj"].astype(f32)
    inv1 = inputs["bn1_gamma"] / np.sqrt(inputs["bn1_var"] + EPS)
    A1 = (inv1 * 0.5).astype(f32)
    B1 = (inputs["bn1_beta"] - inv1 * inputs["bn1_mean"]).astype(f32)
    inv2 = inputs["bn2_gamma"] / np.sqrt(inputs["bn2_var"] + EPS)
    A2 = (inv2 * 0.5).astype(f32)
    B2 = (inputs["bn2_beta"] - inv2 * inputs["bn2_mean"]).astype(f32)
    gam1 = (4.0 * np.sqrt(inputs["fr_x"].reshape(NH) * CH)).astype(f32)
    gam2 = (4.0 * np.sqrt(inputs["fr_attn"].reshape(NH) * NP)).astype(f32)

    # conv output channel permutation: new chan g*128+32e+d -> head 4g+e row d (y1),
    # new chan 384+g*128+32e+d -> y2 of head 4g+e
    perm = np.empty(2 * C, dtype=np.int64)
    for g in range(3):
        for e in range(4):
            h = 4 * g + e
            d = np.arange(32)
            perm[g * 128 + 32 * e + d] = h * 64 + d
            perm[384 + g * 128 + 32 * e + d] = h * 64 + 32 + d

    # wconv tiles: [6 mt][3 kt][128 c][16 ij * 128 o] with permuted out channels
    wc = w_conv[perm]                        # [768, 384, 4, 4]
    # -> [mt, o(128), kt, c(128), i, j]
    wc = wc.reshape(6, 128, 3, 128, 4, 4)
    # [mt][kt][c][ij*128+o]
    wct = wc.transpose(0, 2, 3, 4, 5, 1).reshape(6, 3, 128, 16 * 128)
    wconv_t = np.ascontiguousarray(wct).astype(bf16np)

    # wproj tiles: [128 c, (mt*3+kt)*128 + o]
    wp = w_proj.reshape(3, 128, 3, 128)      # [mt, o, kt, c]
    wpt = wp.transpose(2, 3, 0, 1).reshape(3, 128, 3 * 128)   # [kt][c][mt*128+o]
    # reorder free as (mt*3+kt)*128+o
    wpj = np.empty((128, 9 * 128), dtype=bf16np)
    for mt in range(3):
        for kt in range(3):
            wpj[:, (mt * 3 + kt) * 128:(mt * 3 + kt + 1) * 128] = \
                wpt[kt][:, mt * 128:(mt + 1) * 128].astype(bf16np)

    consts = np.zeros((128, 24), dtype=f32)
    A1p, B1p = A1[perm], B1[perm]
    for mt in range(6):
        consts[:, mt] = A1p[mt * 128:(mt + 1) * 128]
        consts[:, 6 + mt] = B1p[mt * 128:(mt + 1) * 128]
    for p in range(6):
        consts[0:64, 12 + p] = gam1[2 * p]
        consts[64:128, 12 + p] = gam1[2 * p + 1]
    for g in range(3):
        consts[:, 18 + g] = np.repeat(gam2[4 * g:4 * g + 4], 32)
        consts[:, 21 + g] = A2[g * 128:(g + 1) * 128]

    aux = np.zeros((1, 896), dtype=f32)
    aux[0, 0:512] = 1.0
    aux[0, 512:512 + 384] = B2 / A2

    return wconv_t, wpj, consts, aux


def kernel(**inputs):
    inputs = {k: np.asarray(v) for k, v in inputs.items()}
    if "nc" not in _CACHE:
        _CACHE["nc"] = _build_program()
    nc = _CACHE["nc"]

    wconv_t, wpj, consts, aux = _host_prep(inputs)
    x = inputs["x"].astype(np.float32)          # [T, B, C, H, W]

    in_maps = []
    for b in range(8):
        xb = np.ascontiguousarray(x[:, b].reshape(T, CT, 128, N))
        in_maps.append({"x": xb, "wconv": wconv_t, "wproj": wpj,
                        "consts": consts, "aux": aux})

    res = run_bass_kernel_spmd(nc, in_maps, list(range(8)))

    out = np.empty((T, B, C, H, W), dtype=np.float32)
    for b in range(8):
        yb = res.results[b]["y"]               # [T, CT, 128, N]
        out[:, b] = yb.reshape(T, C, H, W)
    return out



# revision 2
# speedup vs baseline: 1.6102x; 1.6102x over previous
"""Trainium2 Bass kernel v2 for nn_DSSA (spiking self-attention block).

Sharding: data-parallel over B — core b handles batch element b (B=8, 8 cores).

v2 design (validated numerics: ~2.3e-3 norm-rel, dominated by bf16 output):
- x uploaded bf16 with columns permuted j-major (cInt = (j>>1)*512 + (j&1)*256
  + i*64 + hp*8 + wp); y returned bf16 and un-permuted on host.
- conv weights fp8e4m3 (x32), patchify conv = 144 fp8 DoubleRow matmuls
  (K=256 per pass, 0.5 cyc/row) accumulating over (ct, j-pair, i).
- x-LIF spikes {0,2} written straight into the j-major fp8 sx tiles (scattered
  output AP, cost-neutral); MM1 consumes the same fp8 tiles with bf16 lhsT
  (mixed-dtype matmul, HW-verified).
- MM1 block-diagonal: one [64,128] bf16 lhsT per head-pair -> 2 matmuls per
  (t,p) instead of 4; attn state Gat is added in-PSUM by a 0.5*I identity
  matmul prepended to the accumulation group.
- attn evac on ACT, spikes on DVE tensor_scalar (4x mode), resets on Pool STT.
- out-LIF: evac+state-add fused in one DVE STT from PSUM; so spikes fp8 into
  a single [128, 4096] tile (3 g-slices + ones slice) for the proj DoubleRow.
- proj: 4 fp8 DR matmuls per (t,mt) with BN2 bias riding as a k-tile row.
- epilogue: DVE STT (pj*A2c + x) -> bf16 y, 12 output DMAs.
"""

import numpy as np
import ml_dtypes

import concourse.bacc as bacc
import concourse.mybir as mybir
from concourse.tile import TileContext
from concourse.bass_utils import run_bass_kernel_spmd

bf16np = ml_dtypes.bfloat16
f8np = ml_dtypes.float8_e4m3fn
F32 = mybir.dt.float32
BF16 = mybir.dt.bfloat16
FP8 = mybir.dt.float8e4
ALU = mybir.AluOpType
ACTF = mybir.ActivationFunctionType
DR = mybir.MatmulPerfMode.DoubleRow

T, B, C, H, W = 4, 8, 384, 32, 32
NH, CH, P = 12, 32, 4
NP = 64                      # patches
N = H * W                    # 1024
CT = C // 128                # 3 channel tiles
EPS = 1e-5
SC = 32.0                    # conv weight fp8 scale
SP = 16.0                    # proj weight fp8 scale

_CACHE = {}


def _build_program():
    nc = bacc.Bacc("TRN2", target_bir_lowering=False)

    x_in = nc.declare_dram_parameter("x", [T, CT, 128, N], BF16, isOutput=False)
    wconv = nc.declare_dram_parameter("wconv", [6, CT, 128, 2048], FP8, isOutput=False)
    wproj = nc.declare_dram_parameter("wproj", [128, 1536], FP8, isOutput=False)
    consts = nc.declare_dram_parameter("consts", [128, 24], F32, isOutput=False)
    ident = nc.declare_dram_parameter("ident", [128, 128], BF16, isOutput=False)
    ones8 = nc.declare_dram_parameter("ones8", [128, N], FP8, isOutput=False)
    y_out = nc.declare_dram_parameter("y", [T, CT, 128, N], BF16, isOutput=True)

    # consts cols: 0-5 A1p(mt), 6-11 B1p(mt), 12-17 gam1(pair), 18-20 gam2(g),
    # 21-23 A2c(mt)
    with TileContext(nc) as tc:
        with tc.tile_pool(name="sb", bufs=1) as sb:
            cst = sb.tile([128, 24], F32, tag="cst", name="cst")
            nc.sync.dma_start(cst[:], consts[:])
            wpj = sb.tile([128, 1536], FP8, tag="wpj", name="wpj")
            nc.sync.dma_start(wpj[:], wproj[:])
            id05 = sb.tile([128, 128], BF16, tag="id05", name="id05")
            nc.sync.dma_start(id05[:], ident[:])

            # persistent tiles
            xt = [[sb.tile([128, N], BF16, tag=f"x{t}{ct}", name=f"x{t}{ct}")
                   for ct in range(CT)] for t in range(T)]
            xv = [sb.tile([128, N], BF16, tag=f"xv{ct}", name=f"xv{ct}")
                  for ct in range(CT)]
            sx = [sb.tile([128, T * N], FP8, tag=f"sx{ct}", name=f"sx{ct}")
                  for ct in range(CT)]
            Gat = [sb.tile([128, N], BF16, tag=f"Gat{p}", name=f"Gat{p}")
                   for p in range(6)]
            Got = [sb.tile([128, N], BF16, tag=f"Got{g}", name=f"Got{g}")
                   for g in range(CT)]
            y1c = [sb.tile([128, T * NP], BF16, tag=f"y1c{g}", name=f"y1c{g}") for g in range(CT)]
            y2c = [sb.tile([128, T * NP], BF16, tag=f"y2c{g}", name=f"y2c{g}") for g in range(CT)]
            so4 = [sb.tile([128, 4 * N], FP8, tag=f"so4_{t}", name=f"so4_{t}")
                   for t in range(T)]
            saT = [[sb.tile([128, N], BF16, tag=f"sa{t}_{p}", name=f"sa{t}_{p}")
                    for p in range(6)] for t in range(T)]
            # MM1 block-diag lhsT (ping-pong by t parity), MM2 lhsT ditto
            L1 = [[sb.tile([128, 128], BF16, tag=f"L1_{s}{g}", name=f"L1_{s}{g}")
                   for g in range(CT)] for s in range(2)]
            L2 = [[sb.tile([128, 64], BF16, tag=f"L2_{s}{p}", name=f"L2_{s}{p}")
                   for p in range(6)] for s in range(T)]
            def emit_memsets():
                for s in range(2):
                    for g in range(CT):
                        nc.vector.memset(L1[s][g][:], 0.0)
                for s in range(T):
                    for p in range(6):
                        nc.vector.memset(L2[s][p][:], 0.0)

            def emit_ones():
                for t in range(T):
                    nc.sync.dma_start(so4[t][:, 3 * N:4 * N], ones8[:])

            # j-major views of sx: free = jb*2048 + jp*1024 + i*256 + t*64 + n
            sxc = [sx[ct].rearrange("c (jb jp i m) -> c jb jp i m",
                                    jb=2, jp=2, i=4, m=256) for ct in range(CT)]
            sxm = [sx[ct].rearrange("c (jb jp i t n) -> c jb jp i t n",
                                    jb=2, jp=2, i=4, t=4, n=64) for ct in range(CT)]

            # ---- x-LIF (per ct, t-sequential): U>=2 spikes {0,2} fp8,
            #      scattered into the j-major sx tile ----
            def emit_xlif_all(xl):
                # t-major emission; STT is DVE-only on HW, so the recurrence
                # uses a pre-halved state: m = (U<2)*0.5 (DVE TS, 4x), then
                # xv = m*U (TT), then U' = xv + x (TT). Chains alternate
                # engines per ct to pace with the t-major x DMA arrivals.
                ch = [nc.gpsimd, nc.vector, nc.gpsimd]
                for t in range(T):
                    for ct in range(CT):
                        if t == 0:
                            U = xt[0][ct][:]
                        else:
                            Ut = xl.tile([128, N], BF16, tag="xu",
                                         name=f"xu{ct}{t}")
                            ch[ct].tensor_tensor(
                                Ut[:], xv[ct][:], xt[t][ct][:], ALU.add)
                            U = Ut[:]
                        nc.vector.tensor_scalar(
                            sxm[ct][:, :, :, :, t, :], U, 2.0, 2.0,
                            ALU.is_ge, ALU.mult)
                        if t < T - 1:
                            m = xl.tile([128, N], BF16, tag="xm",
                                        name=f"xm{ct}{t}")
                            nc.vector.tensor_scalar(
                                m[:], U, 2.0, 0.5, ALU.is_lt, ALU.mult)
                            ch[ct].tensor_tensor(
                                xv[ct][:], m[:], U, ALU.mult)

            with tc.tile_pool(name="cw", bufs=4) as cw, \
                 tc.tile_pool(name="xl", bufs=9) as xl, \
                 tc.tile_pool(name="tl", bufs=2) as tl, \
                 tc.tile_pool(name="pmp", bufs=2, space="PSUM") as pmp:

                def emit_bn1(mt, pc):
                    dst = y1c[mt] if mt < 3 else y2c[mt - 3]
                    nc.scalar.activation(dst[:], pc[:], ACTF.Identity,
                                         bias=cst[:, 6 + mt:7 + mt],
                                         scale=cst[:, mt:mt + 1])

                def emit_conv_mt(wt, ct, pc, first, last):
                    # 8 fp8 DoubleRow matmuls: (jp, i); ktile pair = jb
                    wv = wt.rearrange("c (jp i jb o) -> c jp i jb o",
                                      jp=2, i=4, jb=2, o=128)
                    for jp in range(2):
                        for i in range(4):
                            nc.tensor.matmul(
                                pc[:], wv[:, jp, i], sxc[ct][:, :, jp, i],
                                start=(first and jp == 0 and i == 0),
                                stop=(last and jp == 1 and i == 3),
                                perf_mode=DR, skip_group_check=True)

                def emit_l1(t, p):
                    g, jj = p // 2, p % 2
                    Ls = L1[t % 2][g]
                    r0 = 64 * jj
                    eng = nc.vector
                    eng.tensor_copy(
                        Ls[r0:r0 + 32, 0:64], y1c[g][r0:r0 + 32,
                                                     t * 64:(t + 1) * 64])
                    eng.tensor_copy(
                        Ls[r0 + 32:r0 + 64, 64:128], y1c[g][r0 + 32:r0 + 64,
                                                            t * 64:(t + 1) * 64])

                def emit_ltrans_pair(t, p):
                        g, jj = p // 2, p % 2
                        Ls = L2[t][p]
                        r0 = 64 * jj
                        c0 = t * 64
                        # hA [32 d, 64 p] -> L2[0:64, 0:32]; hB -> L2[64:128, 32:64]
                        nc.vector.transpose(
                            Ls[0:32, 0:32], y2c[g][r0:r0 + 32, c0:c0 + 32])
                        nc.vector.transpose(
                            Ls[32:64, 0:32], y2c[g][r0:r0 + 32, c0 + 32:c0 + 64])
                        nc.vector.transpose(
                            Ls[64:96, 32:64], y2c[g][r0 + 32:r0 + 64, c0:c0 + 32])
                        nc.vector.transpose(
                            Ls[96:128, 32:64], y2c[g][r0 + 32:r0 + 64,
                                                      c0 + 32:c0 + 64])

                sa = {}

                def emit_attn(t, p):
                    g, jj = p // 2, p % 2
                    rA = 64 * jj
                    pm = pmp.tile([128, N], F32, tag="pm", name=f"pm{t}_{p}")
                    for nh in range(2):
                        if t > 0:
                            nc.tensor.matmul(
                                pm[:, nh * 512:(nh + 1) * 512], id05[:],
                                Gat[p][:, nh * 512:(nh + 1) * 512],
                                start=True, stop=False, skip_group_check=True)
                        nc.tensor.matmul(
                            pm[:, nh * 512:(nh + 1) * 512],
                            L1[t % 2][g][rA:rA + 64, :],
                            sxm[g][rA:rA + 64, nh, :, :, t, :],
                            start=(t == 0), stop=True, skip_group_check=True)
                    Ua = tl.tile([128, N], BF16, tag="Ua", name=f"Ua{t}{p}")
                    nc.scalar.copy(Ua[:], pm[:])
                    sat = saT[t][p]
                    nc.vector.tensor_scalar(
                        sat[:], Ua[:], cst[:, 12 + p:13 + p], 2.0,
                        ALU.is_ge, ALU.mult)
                    sa[(t, p)] = sat
                    if t < T - 1:
                        m = tl.tile([128, N], BF16, tag="am", name=f"am{t}{p}")
                        nc.vector.tensor_scalar(
                            m[:], Ua[:], cst[:, 12 + p:13 + p], 0.5,
                            ALU.is_lt, ALU.mult)
                        nc.gpsimd.tensor_tensor(Gat[p][:], m[:], Ua[:],
                                                ALU.mult)

                pools = {}

                def emit_mm2_outlif(t, g):
                        Uo = tl.tile([128, N], BF16, tag="Uo", name=f"Uo{t}{g}")
                        for nh in range(2):
                            po = pools["pop"].tile([128, 512], F32, tag="po",
                                                   name=f"po{t}{g}{nh}")
                            for jj in range(2):
                                p = 2 * g + jj
                                nc.tensor.matmul(
                                    po[64 * jj:64 * jj + 64, :],
                                    L2[t][p][:, 0:64],
                                    sa[(t, p)][:, nh * 512:(nh + 1) * 512],
                                    start=True, stop=True,
                                    tile_position=(0, 64 * jj))
                            if t == 0:
                                nc.scalar.copy(Uo[:, nh * 512:(nh + 1) * 512],
                                               po[:])
                            else:
                                nc.vector.tensor_tensor(
                                    Uo[:, nh * 512:(nh + 1) * 512],
                                    Got[g][:, nh * 512:(nh + 1) * 512],
                                    po[:], ALU.add)
                        seng = nc.gpsimd
                        seng.tensor_scalar(
                            so4[t][:, g * N:(g + 1) * N], Uo[:],
                            cst[:, 18 + g:19 + g], 2.0, ALU.is_ge, ALU.mult)
                        if t < T - 1:
                            m = tl.tile([128, N], BF16, tag="om",
                                        name=f"om{t}{g}")
                            nc.vector.tensor_scalar(
                                m[:], Uo[:], cst[:, 18 + g:19 + g], 0.5,
                                ALU.is_lt, ALU.mult)
                            nc.gpsimd.tensor_tensor(Got[g][:], m[:], Uo[:],
                                                    ALU.mult)

                sov4 = [so4[t].rearrange("c (g n) -> c g n", g=4, n=N)
                        for t in range(T)]
                wpv = wpj.rearrange("c (mt dr kt o) -> c mt dr kt o",
                                   mt=3, dr=2, kt=2, o=128)

                def emit_proj_epi(t, mt):
                        of = tl.tile([128, N], BF16, tag="of", name=f"of{t}{mt}")
                        act_route = mt > 0
                        if act_route:
                            of1 = tl.tile([128, N], BF16, tag="of1",
                                          name=f"of1_{t}{mt}")
                        for nh in range(2):
                            pj = pools["pjp"].tile([128, 512], F32, tag="pj",
                                                   name=f"pj{t}{mt}{nh}")
                            for dr in range(2):
                                nc.tensor.matmul(
                                    pj[:], wpv[:, mt, dr],
                                    sov4[t][:, 2 * dr:2 * dr + 2,
                                            nh * 512:(nh + 1) * 512],
                                    start=(dr == 0), stop=(dr == 1),
                                    perf_mode=DR, skip_group_check=True)
                            if act_route:
                                nc.scalar.activation(
                                    of1[:, nh * 512:(nh + 1) * 512], pj[:],
                                    ACTF.Copy, bias=0.0,
                                    scale=cst[:, 21 + mt:22 + mt])
                            else:
                                nc.vector.scalar_tensor_tensor(
                                    of[:, nh * 512:(nh + 1) * 512], pj[:],
                                    cst[:, 21 + mt:22 + mt],
                                    xt[t][mt][:, nh * 512:(nh + 1) * 512],
                                    ALU.mult, ALU.add)
                        if act_route:
                            nc.vector.tensor_tensor(
                                of[:], of1[:], xt[t][mt][:], ALU.add)
                        nc.sync.dma_start(y_out[t, mt], of[:])

                # ================= schedule =================
                with tc.tile_pool(name="cp1", bufs=1, space="PSUM") as cp1:
                    pcw1 = [cp1.tile([128, T * NP], F32, tag=f"pcw1_{m}",
                                     name=f"pcw1_{m}") for m in range(3)]
                    # t-major x DMAs so all three chains start early
                    for t in range(T):
                        for ct in range(CT):
                            nc.sync.dma_start(xt[t][ct][:], x_in[t, ct])
                    emit_xlif_all(xl)
                    emit_memsets()
                    for mi, mt in enumerate((0, 1, 2)):
                        for ct in range(CT):
                            wt = cw.tile([128, 2048], FP8, tag="wc",
                                         name=f"w1c{mt}{ct}")
                            nc.sync.dma_start(wt[:], wconv[mt, ct])
                            emit_conv_mt(wt, ct, pcw1[mi],
                                         first=(ct == 0), last=(ct == 2))
                        emit_bn1(mt, pcw1[mi])
                        emit_l1(0, 2 * mi)
                        emit_l1(0, 2 * mi + 1)
                        emit_attn(0, 2 * mi)
                        emit_attn(0, 2 * mi + 1)
                    with tc.tile_pool(name="cp2", bufs=1, space="PSUM") as cp2:
                        for g in range(CT):
                            mt = 3 + g
                            pc = cp2.tile([128, T * NP], F32, tag="pc",
                                          name=f"pc{g}")
                            for ct in range(CT):
                                wt = cw.tile([128, 2048], FP8, tag="wc",
                                             name=f"w2c{g}{ct}")
                                nc.sync.dma_start(wt[:], wconv[mt, ct])
                                emit_conv_mt(wt, ct, pc,
                                             first=(ct == 0), last=(ct == 2))
                            emit_bn1(mt, pc)
                            for tt in range(T):
                                emit_ltrans_pair(tt, 2 * g)
                                emit_ltrans_pair(tt, 2 * g + 1)
                emit_ones()
                with tc.tile_pool(name="pop", bufs=2, space="PSUM") as pop_, \
                     tc.tile_pool(name="pjp", bufs=2, space="PSUM") as pjp_:
                    pools["pop"] = pop_
                    pools["pjp"] = pjp_
                    # software pipeline: attn(t+1) ahead of mm2(t); epi lags 2
                    for p in range(6):
                        emit_l1(1, p)
                        emit_attn(1, p)
                    for g in range(CT):
                        emit_mm2_outlif(0, g)
                    for t in range(2, T):
                        for p in range(6):
                            emit_l1(t, p)
                            emit_attn(t, p)
                        for g in range(CT):
                            emit_proj_epi(t - 2, g)
                            emit_mm2_outlif(t - 1, g)
                    for g in range(CT):
                        emit_proj_epi(T - 2, g)
                        emit_mm2_outlif(T - 1, g)
                    for g in range(CT):
                        emit_proj_epi(T - 1, g)
    nc.compile()
    return nc


def _host_prep(inputs):
    f32 = np.float32
    w_conv = inputs["w_conv"].astype(f32)
    w_proj = inputs["w_proj"].astype(f32)
    inv1 = inputs["bn1_gamma"] / np.sqrt(inputs["bn1_var"] + EPS)
    B1 = inputs["bn1_beta"] - inv1 * inputs["bn1_mean"]
    inv2 = inputs["bn2_gamma"] / np.sqrt(inputs["bn2_var"] + EPS)
    B2 = inputs["bn2_beta"] - inv2 * inputs["bn2_mean"]
    gam1 = (4.0 * np.sqrt(inputs["fr_x"].reshape(NH) * CH)).astype(f32)
    gam2 = (4.0 * np.sqrt(inputs["fr_attn"].reshape(NH) * NP)).astype(f32)

    # conv output-channel permutation: tile row g*128 + 64*jj + 32*hh + d
    # holds head (4g+2jj+hh): y1 rows from chan h*64+d, y2 from h*64+32+d
    perm = np.empty(2 * C, np.int64)
    d = np.arange(32)
    for g in range(3):
        for jj in range(2):
            for hh in range(2):
                h = 4 * g + 2 * jj + hh
                r = g * 128 + 64 * jj + 32 * hh + d
                perm[r] = h * 64 + d
                perm[384 + r] = h * 64 + 32 + d

    # wconv8 [6, 3, 128, 2048]: free = jp*1024 + i*256 + jb*128 + o
    wc6 = (w_conv[perm] * SC).reshape(6, 128, CT, 128, 4, 4)  # [mt,o,ct,c,i,j]
    w8 = np.zeros((6, CT, 128, 2048), f8np)
    for jp in range(2):
        for i in range(4):
            for jb in range(2):
                j = 2 * jb + jp
                col = jp * 1024 + i * 256 + jb * 128
                w8[:, :, :, col:col + 128] = \
                    wc6[:, :, :, :, i, j].transpose(0, 2, 3, 1).astype(f8np)

    # wproj8 [128, 1536]: free = mt*512 + dr*256 + kt*128 + o
    r = np.arange(128)
    bias_p = (B2 * (2.0 * SP) / inv2).astype(f32)
    wpj8 = np.zeros((128, 1536), f8np)
    for mt in range(3):
        for dridx in range(2):
            for kt in range(2):
                col = mt * 512 + dridx * 256 + kt * 128
                gidx = dridx * 2 + kt
                if gidx < 3:
                    ch = 32 * (4 * gidx + 2 * (r >> 6) + ((r >> 5) & 1)) + (r & 31)
                    wpj8[:, col:col + 128] = \
                        (SP * w_proj[mt * 128:(mt + 1) * 128, ch]).T.astype(f8np)
                else:
                    blk = np.zeros((128, 128), f32)
                    blk[0, :] = bias_p[mt * 128:(mt + 1) * 128]
                    wpj8[:, col:col + 128] = blk.astype(f8np)

    consts = np.zeros((128, 24), f32)
    A1p = inv1[perm] / (2.0 * SC)
    B1p = B1[perm]
    for mt in range(6):
        consts[:, mt] = A1p[mt * 128:(mt + 1) * 128]
        consts[:, 6 + mt] = B1p[mt * 128:(mt + 1) * 128]
    for p in range(6):
        consts[0:64, 12 + p] = gam1[2 * p]
        consts[64:128, 12 + p] = gam1[2 * p + 1]
    for g in range(3):
        consts[:, 18 + g] = np.repeat(gam2[4 * g:4 * g + 4], 32)
        consts[:, 21 + g] = (inv2 / (2.0 * SP))[g * 128:(g + 1) * 128]

    ident05 = np.eye(128, dtype=f32).astype(bf16np)

    # column permutation: cInt(n) = (j>>1)*512 + (j&1)*256 + i*64 + hp*8 + wp
    n = np.arange(N)
    hp, rr = n >> 7, n & 127
    ii = rr >> 5
    r2 = rr & 31
    wp = r2 >> 2
    jj = r2 & 3
    cInt = ((jj >> 1) * 512 + (jj & 1) * 256 + ii * 64 + hp * 8 + wp)
    ninv = np.empty(N, np.int64)
    ninv[cInt] = n

    return w8, wpj8, consts, ident05, cInt, ninv


def kernel(**inputs):
    inputs = {k: np.asarray(v) for k, v in inputs.items()}
    if "nc" not in _CACHE:
        _CACHE["nc"] = _build_program()
    nc = _CACHE["nc"]

    w8, wpj8, consts, ident05, cInt, ninv = _host_prep(inputs)
    x = inputs["x"].astype(np.float32).reshape(T, B, C, N)
    xp = np.ascontiguousarray(x[..., ninv]).astype(bf16np)  # j-major cols

    in_maps = []
    for b in range(8):
        xb = np.ascontiguousarray(xp[:, b].reshape(T, CT, 128, N))
        in_maps.append({"x": xb, "wconv": w8, "wproj": wpj8,
                        "consts": consts, "ident": ident05,
                        "ones8": np.ones((128, N), f8np)})

    res = run_bass_kernel_spmd(nc, in_maps, list(range(8)))

    out = np.empty((T, B, C, H, W), dtype=np.float32)
    for b in range(8):
        yb = res.results[b]["y"].astype(np.float32)     # [T, CT, 128, N]
        out[:, b] = yb.reshape(T, C, N)[..., cInt].reshape(T, C, H, W)
    return out


# revision 3
# speedup vs baseline: 1.6199x; 1.0060x over previous
"""Trainium2 Bass kernel v2 for nn_DSSA (spiking self-attention block).

Sharding: data-parallel over B — core b handles batch element b (B=8, 8 cores).

v2 design (validated numerics: ~2.3e-3 norm-rel, dominated by bf16 output):
- x uploaded bf16 with columns permuted j-major (cInt = (j>>1)*512 + (j&1)*256
  + i*64 + hp*8 + wp); y returned bf16 and un-permuted on host.
- conv weights fp8e4m3 (x32), patchify conv = 144 fp8 DoubleRow matmuls
  (K=256 per pass, 0.5 cyc/row) accumulating over (ct, j-pair, i).
- x-LIF spikes {0,2} written straight into the j-major fp8 sx tiles (scattered
  output AP, cost-neutral); MM1 consumes the same fp8 tiles with bf16 lhsT
  (mixed-dtype matmul, HW-verified).
- MM1 block-diagonal: one [64,128] bf16 lhsT per head-pair -> 2 matmuls per
  (t,p) instead of 4; attn state Gat is added in-PSUM by a 0.5*I identity
  matmul prepended to the accumulation group.
- attn evac on ACT, spikes on DVE tensor_scalar (4x mode), resets on Pool STT.
- out-LIF: evac+state-add fused in one DVE STT from PSUM; so spikes fp8 into
  a single [128, 4096] tile (3 g-slices + ones slice) for the proj DoubleRow.
- proj: 4 fp8 DR matmuls per (t,mt) with BN2 bias riding as a k-tile row.
- epilogue: DVE STT (pj*A2c + x) -> bf16 y, 12 output DMAs.
"""

import numpy as np
import ml_dtypes

import concourse.bacc as bacc
import concourse.mybir as mybir
from concourse.tile import TileContext
from concourse.bass_utils import run_bass_kernel_spmd

bf16np = ml_dtypes.bfloat16
f8np = ml_dtypes.float8_e4m3fn
F32 = mybir.dt.float32
BF16 = mybir.dt.bfloat16
FP8 = mybir.dt.float8e4
ALU = mybir.AluOpType
ACTF = mybir.ActivationFunctionType
DR = mybir.MatmulPerfMode.DoubleRow

T, B, C, H, W = 4, 8, 384, 32, 32
NH, CH, P = 12, 32, 4
NP = 64                      # patches
N = H * W                    # 1024
CT = C // 128                # 3 channel tiles
EPS = 1e-5
SC = 32.0                    # conv weight fp8 scale
SP = 16.0                    # proj weight fp8 scale

_CACHE = {}


def _build_program():
    nc = bacc.Bacc("TRN2", target_bir_lowering=False)

    x_in = nc.declare_dram_parameter("x", [T, CT, 128, N], BF16, isOutput=False)
    wconv = nc.declare_dram_parameter("wconv", [6, CT, 128, 2048], FP8, isOutput=False)
    wproj = nc.declare_dram_parameter("wproj", [128, 1536], FP8, isOutput=False)
    consts = nc.declare_dram_parameter("consts", [128, 24], F32, isOutput=False)
    ident = nc.declare_dram_parameter("ident", [128, 128], BF16, isOutput=False)
    ones8 = nc.declare_dram_parameter("ones8", [128, N], FP8, isOutput=False)
    y_out = nc.declare_dram_parameter("y", [T, CT, 128, N], BF16, isOutput=True)

    # consts cols: 0-5 A1p(mt), 6-11 B1p(mt), 12-17 gam1(pair), 18-20 gam2(g),
    # 21-23 A2c(mt)
    with TileContext(nc) as tc:
        with tc.tile_pool(name="sb", bufs=1) as sb:
            cst = sb.tile([128, 24], F32, tag="cst", name="cst")
            nc.sync.dma_start(cst[:], consts[:])
            wpj = sb.tile([128, 1536], FP8, tag="wpj", name="wpj")
            nc.sync.dma_start(wpj[:], wproj[:])
            id05 = sb.tile([128, 128], BF16, tag="id05", name="id05")
            nc.sync.dma_start(id05[:], ident[:])

            # persistent tiles
            xt = [[sb.tile([128, N], BF16, tag=f"x{t}{ct}", name=f"x{t}{ct}")
                   for ct in range(CT)] for t in range(T)]
            xv = [sb.tile([128, N], BF16, tag=f"xv{ct}", name=f"xv{ct}")
                  for ct in range(CT)]
            sx = [sb.tile([128, T * N], FP8, tag=f"sx{ct}", name=f"sx{ct}")
                  for ct in range(CT)]
            Gat = [sb.tile([128, 2 * N], BF16, tag=f"Gat{g}", name=f"Gat{g}")
                   for g in range(CT)]
            Got = [sb.tile([128, N], BF16, tag=f"Got{g}", name=f"Got{g}")
                   for g in range(CT)]
            y1c = [sb.tile([128, T * NP], BF16, tag=f"y1c{g}", name=f"y1c{g}") for g in range(CT)]
            y2c = [sb.tile([128, T * NP], BF16, tag=f"y2c{g}", name=f"y2c{g}") for g in range(CT)]
            so4 = [sb.tile([128, 4 * N], FP8, tag=f"so4_{t}", name=f"so4_{t}")
                   for t in range(T)]
            saT = [[sb.tile([128, 2 * N], BF16, tag=f"sa{t}_{g}", name=f"sa{t}_{g}")
                    for g in range(CT)] for t in range(T)]
            # MM1 block-diag lhsT (ping-pong by t parity), MM2 lhsT ditto
            L1 = [[sb.tile([128, 128], BF16, tag=f"L1_{s}{g}", name=f"L1_{s}{g}")
                   for g in range(CT)] for s in range(2)]
            L2 = [[sb.tile([128, 64], BF16, tag=f"L2_{s}{p}", name=f"L2_{s}{p}")
                   for p in range(6)] for s in range(T)]
            def emit_memsets():
                for s in range(2):
                    for g in range(CT):
                        nc.gpsimd.memset(L1[s][g][:], 0.0)
                for s in range(T):
                    for p in range(6):
                        nc.gpsimd.memset(L2[s][p][:], 0.0)

            def emit_ones():
                for t in range(T):
                    nc.sync.dma_start(so4[t][:, 3 * N:4 * N], ones8[:])

            # j-major views of sx: free = jb*2048 + jp*1024 + i*256 + t*64 + n
            sxc = [sx[ct].rearrange("c (jb jp i m) -> c jb jp i m",
                                    jb=2, jp=2, i=4, m=256) for ct in range(CT)]
            sxm = [sx[ct].rearrange("c (jb jp i t n) -> c jb jp i t n",
                                    jb=2, jp=2, i=4, t=4, n=64) for ct in range(CT)]

            # ---- x-LIF (per ct, t-sequential): U>=2 spikes {0,2} fp8,
            #      scattered into the j-major sx tile ----
            def emit_xlif_all(xl):
                # t-major emission; STT is DVE-only on HW, so the recurrence
                # uses a pre-halved state: m = (U<2)*0.5 (DVE TS, 4x), then
                # xv = m*U (TT), then U' = xv + x (TT). Chains alternate
                # engines per ct to pace with the t-major x DMA arrivals.
                ch = [nc.gpsimd, nc.vector, nc.gpsimd]
                for t in range(T):
                    for ct in range(CT):
                        if t == 0:
                            U = xt[0][ct][:]
                        else:
                            Ut = xl.tile([128, N], BF16, tag="xu",
                                         name=f"xu{ct}{t}")
                            ch[ct].tensor_tensor(
                                Ut[:], xv[ct][:], xt[t][ct][:], ALU.add)
                            U = Ut[:]
                        nc.vector.tensor_scalar(
                            sxm[ct][:, :, :, :, t, :], U, 2.0, 2.0,
                            ALU.is_ge, ALU.mult)
                        if t < T - 1:
                            m = xl.tile([128, N], BF16, tag="xm",
                                        name=f"xm{ct}{t}")
                            nc.vector.tensor_scalar(
                                m[:], U, 2.0, 0.5, ALU.is_lt, ALU.mult)
                            ch[ct].tensor_tensor(
                                xv[ct][:], m[:], U, ALU.mult)

            with tc.tile_pool(name="cw", bufs=4) as cw, \
                 tc.tile_pool(name="xl", bufs=4) as xl, \
                 tc.tile_pool(name="tl", bufs=2) as tl, \
                 tc.tile_pool(name="pmp", bufs=2, space="PSUM") as pmp:

                def emit_bn1(mt, pc):
                    dst = y1c[mt] if mt < 3 else y2c[mt - 3]
                    nc.scalar.activation(dst[:], pc[:], ACTF.Identity,
                                         bias=cst[:, 6 + mt:7 + mt],
                                         scale=cst[:, mt:mt + 1])

                def emit_conv_mt(wt, ct, pc, first, last):
                    # 8 fp8 DoubleRow matmuls: (jp, i); ktile pair = jb
                    wv = wt.rearrange("c (jp i jb o) -> c jp i jb o",
                                      jp=2, i=4, jb=2, o=128)
                    for jp in range(2):
                        for i in range(4):
                            nc.tensor.matmul(
                                pc[:], wv[:, jp, i], sxc[ct][:, :, jp, i],
                                start=(first and jp == 0 and i == 0),
                                stop=(last and jp == 1 and i == 3),
                                perf_mode=DR, skip_group_check=True)

                def emit_l1(t, p):
                    g, jj = p // 2, p % 2
                    Ls = L1[t % 2][g]
                    r0 = 64 * jj
                    eng = nc.gpsimd
                    eng.tensor_copy(
                        Ls[r0:r0 + 32, 0:64], y1c[g][r0:r0 + 32,
                                                     t * 64:(t + 1) * 64])
                    eng.tensor_copy(
                        Ls[r0 + 32:r0 + 64, 64:128], y1c[g][r0 + 32:r0 + 64,
                                                            t * 64:(t + 1) * 64])

                def emit_ltrans_pair(t, p):
                        g, jj = p // 2, p % 2
                        Ls = L2[t][p]
                        r0 = 64 * jj
                        c0 = t * 64
                        # hA [32 d, 64 p] -> L2[0:64, 0:32]; hB -> L2[64:128, 32:64]
                        nc.vector.transpose(
                            Ls[0:32, 0:32], y2c[g][r0:r0 + 32, c0:c0 + 32])
                        nc.vector.transpose(
                            Ls[32:64, 0:32], y2c[g][r0:r0 + 32, c0 + 32:c0 + 64])
                        nc.vector.transpose(
                            Ls[64:96, 32:64], y2c[g][r0 + 32:r0 + 64, c0:c0 + 32])
                        nc.vector.transpose(
                            Ls[96:128, 32:64], y2c[g][r0 + 32:r0 + 64,
                                                      c0 + 32:c0 + 64])

                def emit_attn(t, g):
                    # per-pair [1024] tiles; thresholds folded into BN1 ->
                    # uniform theta = 1.0
                    Ua = tl.tile([128, 2 * N], BF16, tag="Ua",
                                 name=f"Ua{t}{g}")
                    for jj in range(2):
                        rA = 64 * jj
                        pm = pmp.tile([128, N], F32, tag="pm",
                                      name=f"pm{t}_{g}{jj}")
                        for nh in range(2):
                            cs = slice(jj * N + nh * 512,
                                       jj * N + nh * 512 + 512)
                            if t > 0:
                                nc.tensor.matmul(
                                    pm[:, nh * 512:(nh + 1) * 512], id05[:],
                                    Gat[g][:, cs],
                                    start=True, stop=False,
                                    skip_group_check=True)
                            nc.tensor.matmul(
                                pm[:, nh * 512:(nh + 1) * 512],
                                L1[t % 2][g][rA:rA + 64, :],
                                sxm[g][rA:rA + 64, nh, :, :, t, :],
                                start=(t == 0), stop=True,
                                skip_group_check=True)
                        nc.scalar.copy(Ua[:, jj * N:(jj + 1) * N], pm[:])
                    nc.vector.tensor_scalar(
                        saT[t][g][:], Ua[:], 1.0, 2.0, ALU.is_ge, ALU.mult)
                    if t < T - 1:
                        m = tl.tile([128, 2 * N], BF16, tag="am",
                                    name=f"am{t}{g}")
                        nc.vector.tensor_scalar(
                            m[:], Ua[:], 1.0, 0.5, ALU.is_lt, ALU.mult)
                        nc.gpsimd.tensor_tensor(
                            Gat[g][:], m[:], Ua[:], ALU.mult)

                pools = {}

                def emit_mm2_outlif(t, g):
                        Uo = tl.tile([128, N], BF16, tag="Uo", name=f"Uo{t}{g}")
                        for nh in range(2):
                            po = pools["pop"].tile([128, 512], F32, tag="po",
                                                   name=f"po{t}{g}{nh}")
                            for jj in range(2):
                                p = 2 * g + jj
                                nc.tensor.matmul(
                                    po[64 * jj:64 * jj + 64, :],
                                    L2[t][p][:, 0:64],
                                    saT[t][g][:, jj * N + nh * 512:
                                              jj * N + nh * 512 + 512],
                                    start=True, stop=True,
                                    tile_position=(0, 64 * jj))
                            if t == 0:
                                nc.scalar.copy(Uo[:, nh * 512:(nh + 1) * 512],
                                               po[:])
                            else:
                                nc.vector.tensor_tensor(
                                    Uo[:, nh * 512:(nh + 1) * 512],
                                    Got[g][:, nh * 512:(nh + 1) * 512],
                                    po[:], ALU.add)
                        nc.gpsimd.tensor_scalar(
                            so4[t][:, g * N:(g + 1) * N], Uo[:],
                            1.0, 2.0, ALU.is_ge, ALU.mult)
                        if t < T - 1:
                            m = tl.tile([128, N], BF16, tag="om",
                                        name=f"om{t}{g}")
                            nc.vector.tensor_scalar(
                                m[:], Uo[:], 1.0, 0.5, ALU.is_lt, ALU.mult)
                            nc.gpsimd.tensor_tensor(Got[g][:], m[:], Uo[:],
                                                    ALU.mult)

                sov4 = [so4[t].rearrange("c (g n) -> c g n", g=4, n=N)
                        for t in range(T)]
                wpv = wpj.rearrange("c (mt dr kt o) -> c mt dr kt o",
                                   mt=3, dr=2, kt=2, o=128)

                def emit_proj_epi(t, mt):
                        of = tl.tile([128, N], BF16, tag="of", name=f"of{t}{mt}")
                        act_route = True
                        if act_route:
                            of1 = tl.tile([128, N], BF16, tag="of1",
                                          name=f"of1_{t}{mt}")
                        for nh in range(2):
                            pj = pools["pjp"].tile([128, 512], F32, tag="pj",
                                                   name=f"pj{t}{mt}{nh}")
                            for dr in range(2):
                                nc.tensor.matmul(
                                    pj[:], wpv[:, mt, dr],
                                    sov4[t][:, 2 * dr:2 * dr + 2,
                                            nh * 512:(nh + 1) * 512],
                                    start=(dr == 0), stop=(dr == 1),
                                    perf_mode=DR, skip_group_check=True)
                            if act_route:
                                nc.scalar.activation(
                                    of1[:, nh * 512:(nh + 1) * 512], pj[:],
                                    ACTF.Copy, bias=0.0,
                                    scale=cst[:, 21 + mt:22 + mt])
                            else:
                                nc.vector.scalar_tensor_tensor(
                                    of[:, nh * 512:(nh + 1) * 512], pj[:],
                                    cst[:, 21 + mt:22 + mt],
                                    xt[t][mt][:, nh * 512:(nh + 1) * 512],
                                    ALU.mult, ALU.add)
                        if act_route:
                            nc.vector.tensor_tensor(
                                of[:], of1[:], xt[t][mt][:], ALU.add)
                        nc.sync.dma_start(y_out[t, mt], of[:])

                # ================= schedule =================
                with tc.tile_pool(name="cp1", bufs=1, space="PSUM") as cp1:
                    pcw1 = [cp1.tile([128, T * NP], F32, tag=f"pcw1_{m}",
                                     name=f"pcw1_{m}") for m in range(3)]
                    # t-major x DMAs so all three chains start early
                    for t in range(T):
                        for ct in range(CT):
                            nc.sync.dma_start(xt[t][ct][:], x_in[t, ct])
                    emit_xlif_all(xl)
                    emit_memsets()
                    for mi, mt in enumerate((0, 1, 2)):
                        for ct in range(CT):
                            wt = cw.tile([128, 2048], FP8, tag="wc",
                                         name=f"w1c{mt}{ct}")
                            nc.sync.dma_start(wt[:], wconv[mt, ct])
                            emit_conv_mt(wt, ct, pcw1[mi],
                                         first=(ct == 0), last=(ct == 2))
                        emit_bn1(mt, pcw1[mi])
                        emit_l1(0, 2 * mi)
                        emit_l1(0, 2 * mi + 1)
                        emit_attn(0, mi)
                    with tc.tile_pool(name="cp2", bufs=1, space="PSUM") as cp2:
                        for g in range(CT):
                            mt = 3 + g
                            pc = cp2.tile([128, T * NP], F32, tag="pc",
                                          name=f"pc{g}")
                            for ct in range(CT):
                                wt = cw.tile([128, 2048], FP8, tag="wc",
                                             name=f"w2c{g}{ct}")
                                nc.sync.dma_start(wt[:], wconv[mt, ct])
                                emit_conv_mt(wt, ct, pc,
                                             first=(ct == 0), last=(ct == 2))
                            emit_bn1(mt, pc)
                            for tt in range(T):
                                emit_ltrans_pair(tt, 2 * g)
                                emit_ltrans_pair(tt, 2 * g + 1)
                emit_ones()
                with tc.tile_pool(name="pop", bufs=2, space="PSUM") as pop_, \
                     tc.tile_pool(name="pjp", bufs=2, space="PSUM") as pjp_:
                    pools["pop"] = pop_
                    pools["pjp"] = pjp_
                    # software pipeline: attn(t+1) ahead of mm2(t); epi lags 2
                    for g in range(CT):
                        emit_l1(1, 2 * g)
                        emit_l1(1, 2 * g + 1)
                        emit_attn(1, g)
                    for g in range(CT):
                        emit_mm2_outlif(0, g)
                    for t in range(2, T):
                        for g in range(CT):
                            emit_l1(t, 2 * g)
                            emit_l1(t, 2 * g + 1)
                            emit_attn(t, g)
                        for g in range(CT):
                            emit_proj_epi(t - 2, g)
                            emit_mm2_outlif(t - 1, g)
                    for g in range(CT):
                        emit_proj_epi(T - 2, g)
                        emit_mm2_outlif(T - 1, g)
                    for g in range(CT):
                        emit_proj_epi(T - 1, g)
    nc.compile()
    return nc


def _host_prep(inputs):
    f32 = np.float32
    w_conv = inputs["w_conv"].astype(f32)
    w_proj = inputs["w_proj"].astype(f32)
    inv1 = inputs["bn1_gamma"] / np.sqrt(inputs["bn1_var"] + EPS)
    B1 = inputs["bn1_beta"] - inv1 * inputs["bn1_mean"]
    inv2 = inputs["bn2_gamma"] / np.sqrt(inputs["bn2_var"] + EPS)
    B2 = inputs["bn2_beta"] - inv2 * inputs["bn2_mean"]
    gam1 = (4.0 * np.sqrt(inputs["fr_x"].reshape(NH) * CH)).astype(f32)
    gam2 = (4.0 * np.sqrt(inputs["fr_attn"].reshape(NH) * NP)).astype(f32)

    # conv output-channel permutation: tile row g*128 + 64*jj + 32*hh + d
    # holds head (4g+2jj+hh): y1 rows from chan h*64+d, y2 from h*64+32+d
    perm = np.empty(2 * C, np.int64)
    d = np.arange(32)
    for g in range(3):
        for jj in range(2):
            for hh in range(2):
                h = 4 * g + 2 * jj + hh
                r = g * 128 + 64 * jj + 32 * hh + d
                perm[r] = h * 64 + d
                perm[384 + r] = h * 64 + 32 + d

    # wconv8 [6, 3, 128, 2048]: free = jp*1024 + i*256 + jb*128 + o
    wc6 = (w_conv[perm] * SC).reshape(6, 128, CT, 128, 4, 4)  # [mt,o,ct,c,i,j]
    w8 = np.zeros((6, CT, 128, 2048), f8np)
    for jp in range(2):
        for i in range(4):
            for jb in range(2):
                j = 2 * jb + jp
                col = jp * 1024 + i * 256 + jb * 128
                w8[:, :, :, col:col + 128] = \
                    wc6[:, :, :, :, i, j].transpose(0, 2, 3, 1).astype(f8np)

    # wproj8 [128, 1536]: free = mt*512 + dr*256 + kt*128 + o
    r = np.arange(128)
    bias_p = (B2 * (2.0 * SP) / inv2).astype(f32)
    wpj8 = np.zeros((128, 1536), f8np)
    for mt in range(3):
        for dridx in range(2):
            for kt in range(2):
                col = mt * 512 + dridx * 256 + kt * 128
                gidx = dridx * 2 + kt
                if gidx < 3:
                    ch = 32 * (4 * gidx + 2 * (r >> 6) + ((r >> 5) & 1)) + (r & 31)
                    wpj8[:, col:col + 128] = \
                        (SP * w_proj[mt * 128:(mt + 1) * 128, ch]).T.astype(f8np)
                else:
                    blk = np.zeros((128, 128), f32)
                    blk[0, :] = bias_p[mt * 128:(mt + 1) * 128]
                    wpj8[:, col:col + 128] = blk.astype(f8np)

    consts = np.zeros((128, 24), f32)
    A1p = inv1[perm] / (2.0 * SC)
    B1p = B1[perm]
    # fold LIF thresholds into BN1: y1 rows /gam1[head], y2 rows /gam2[head]
    r = np.arange(128)
    for g in range(3):
        head = 4 * g + 2 * (r >> 6) + ((r >> 5) & 1)
        A1p[g * 128 + r] /= gam1[head]
        B1p[g * 128 + r] /= gam1[head]
        A1p[384 + g * 128 + r] /= gam2[head]
        B1p[384 + g * 128 + r] /= gam2[head]
    for mt in range(6):
        consts[:, mt] = A1p[mt * 128:(mt + 1) * 128]
        consts[:, 6 + mt] = B1p[mt * 128:(mt + 1) * 128]
    for p in range(6):
        consts[0:64, 12 + p] = gam1[2 * p]
        consts[64:128, 12 + p] = gam1[2 * p + 1]
    for g in range(3):
        consts[:, 18 + g] = np.repeat(gam2[4 * g:4 * g + 4], 32)
        consts[:, 21 + g] = (inv2 / (2.0 * SP))[g * 128:(g + 1) * 128]

    ident05 = np.eye(128, dtype=f32).astype(bf16np)

    # column permutation: cInt(n) = (j>>1)*512 + (j&1)*256 + i*64 + hp*8 + wp
    n = np.arange(N)
    hp, rr = n >> 7, n & 127
    ii = rr >> 5
    r2 = rr & 31
    wp = r2 >> 2
    jj = r2 & 3
    cInt = ((jj >> 1) * 512 + (jj & 1) * 256 + ii * 64 + hp * 8 + wp)
    ninv = np.empty(N, np.int64)
    ninv[cInt] = n

    return w8, wpj8, consts, ident05, cInt, ninv


def kernel(**inputs):
    inputs = {k: np.asarray(v) for k, v in inputs.items()}
    if "nc" not in _CACHE:
        _CACHE["nc"] = _build_program()
    nc = _CACHE["nc"]

    w8, wpj8, consts, ident05, cInt, ninv = _host_prep(inputs)
    x = inputs["x"].astype(np.float32).reshape(T, B, C, N)
    xp = np.ascontiguousarray(x[..., ninv]).astype(bf16np)  # j-major cols

    in_maps = []
    for b in range(8):
        xb = np.ascontiguousarray(xp[:, b].reshape(T, CT, 128, N))
        in_maps.append({"x": xb, "wconv": w8, "wproj": wpj8,
                        "consts": consts, "ident": ident05,
                        "ones8": np.ones((128, N), f8np)})

    res = run_bass_kernel_spmd(nc, in_maps, list(range(8)))

    out = np.empty((T, B, C, H, W), dtype=np.float32)
    for b in range(8):
        yb = res.results[b]["y"].astype(np.float32)     # [T, CT, 128, N]
        out[:, b] = yb.reshape(T, C, N)[..., cInt].reshape(T, C, H, W)
    return out


# revision 4
# speedup vs baseline: 1.6849x; 1.0401x over previous
"""Trainium2 Bass kernel v2 for nn_DSSA (spiking self-attention block).

Sharding: data-parallel over B — core b handles batch element b (B=8, 8 cores).

v2 design (validated numerics: ~2.3e-3 norm-rel, dominated by bf16 output):
- x uploaded bf16 with columns permuted j-major (cInt = (j>>1)*512 + (j&1)*256
  + i*64 + hp*8 + wp); y returned bf16 and un-permuted on host.
- conv weights fp8e4m3 (x32), patchify conv = 144 fp8 DoubleRow matmuls
  (K=256 per pass, 0.5 cyc/row) accumulating over (ct, j-pair, i).
- x-LIF spikes {0,2} written straight into the j-major fp8 sx tiles (scattered
  output AP, cost-neutral); MM1 consumes the same fp8 tiles with bf16 lhsT
  (mixed-dtype matmul, HW-verified).
- MM1 block-diagonal: one [64,128] bf16 lhsT per head-pair -> 2 matmuls per
  (t,p) instead of 4; attn state Gat is added in-PSUM by a 0.5*I identity
  matmul prepended to the accumulation group.
- attn evac on ACT, spikes on DVE tensor_scalar (4x mode), resets on Pool STT.
- out-LIF: evac+state-add fused in one DVE STT from PSUM; so spikes fp8 into
  a single [128, 4096] tile (3 g-slices + ones slice) for the proj DoubleRow.
- proj: 4 fp8 DR matmuls per (t,mt) with BN2 bias riding as a k-tile row.
- epilogue: DVE STT (pj*A2c + x) -> bf16 y, 12 output DMAs.
"""

import numpy as np
import ml_dtypes

import concourse.bacc as bacc
import concourse.mybir as mybir
from concourse.tile import TileContext
from concourse.bass_utils import run_bass_kernel_spmd

bf16np = ml_dtypes.bfloat16
f8np = ml_dtypes.float8_e4m3fn
F32 = mybir.dt.float32
BF16 = mybir.dt.bfloat16
FP8 = mybir.dt.float8e4
ALU = mybir.AluOpType
ACTF = mybir.ActivationFunctionType
DR = mybir.MatmulPerfMode.DoubleRow

T, B, C, H, W = 4, 8, 384, 32, 32
NH, CH, P = 12, 32, 4
NP = 64                      # patches
N = H * W                    # 1024
CT = C // 128                # 3 channel tiles
EPS = 1e-5
SC = 32.0                    # conv weight fp8 scale
SP = 16.0                    # proj weight fp8 scale

_CACHE = {}


def _build_program():
    nc = bacc.Bacc("TRN2", target_bir_lowering=False)

    x_in = nc.declare_dram_parameter("x", [T, CT, 128, N], BF16, isOutput=False)
    wconv = nc.declare_dram_parameter("wconv", [6, CT, 128, 2048], FP8, isOutput=False)
    wproj = nc.declare_dram_parameter("wproj", [128, 1536], FP8, isOutput=False)
    consts = nc.declare_dram_parameter("consts", [128, 24], F32, isOutput=False)
    ident = nc.declare_dram_parameter("ident", [128, 128], BF16, isOutput=False)
    ones8 = nc.declare_dram_parameter("ones8", [128, N], FP8, isOutput=False)
    dres = nc.declare_dram_parameter("dres", [128, 3 * 128], BF16, isOutput=False)
    y_out = nc.declare_dram_parameter("y", [T, CT, 128, N], BF16, isOutput=True)

    # consts cols: 0-5 A1p(mt), 6-11 B1p(mt), 12-17 gam1(pair), 18-20 gam2(g),
    # 21-23 A2c(mt)
    with TileContext(nc) as tc:
        with tc.tile_pool(name="sb", bufs=1) as sb:
            cst = sb.tile([128, 24], F32, tag="cst", name="cst")
            nc.sync.dma_start(cst[:], consts[:])
            wpj = sb.tile([128, 1536], FP8, tag="wpj", name="wpj")
            nc.sync.dma_start(wpj[:], wproj[:])
            id05 = sb.tile([128, 128], BF16, tag="id05", name="id05")
            nc.sync.dma_start(id05[:], ident[:])
            drt = sb.tile([128, 3 * 128], BF16, tag="drt", name="drt")
            nc.sync.dma_start(drt[:], dres[:])

            # persistent tiles
            xt = [[sb.tile([128, N], BF16, tag=f"x{t}{ct}", name=f"x{t}{ct}")
                   for ct in range(CT)] for t in range(T)]
            xv = [sb.tile([128, N], BF16, tag=f"xv{ct}", name=f"xv{ct}")
                  for ct in range(CT)]
            sx = [sb.tile([128, T * N], FP8, tag=f"sx{ct}", name=f"sx{ct}")
                  for ct in range(CT)]
            Gat = [sb.tile([128, 2 * N], BF16, tag=f"Gat{g}", name=f"Gat{g}")
                   for g in range(CT)]
            Got = [sb.tile([128, N], BF16, tag=f"Got{g}", name=f"Got{g}")
                   for g in range(CT)]
            y1c = [sb.tile([128, T * NP], BF16, tag=f"y1c{g}", name=f"y1c{g}") for g in range(CT)]
            y2c = [sb.tile([128, T * NP], BF16, tag=f"y2c{g}", name=f"y2c{g}") for g in range(CT)]
            so4 = [sb.tile([128, 4 * N], FP8, tag=f"so4_{t}", name=f"so4_{t}")
                   for t in range(T)]
            saT = [[sb.tile([128, 2 * N], BF16, tag=f"sa{t}_{g}", name=f"sa{t}_{g}")
                    for g in range(CT)] for t in range(T)]
            # MM1 block-diag lhsT (ping-pong by t parity), MM2 lhsT ditto
            L1 = [[sb.tile([128, 128], BF16, tag=f"L1_{s}{g}", name=f"L1_{s}{g}")
                   for g in range(CT)] for s in range(2)]
            L2 = [[sb.tile([128, 64], BF16, tag=f"L2_{s}{p}", name=f"L2_{s}{p}")
                   for p in range(6)] for s in range(T)]
            def emit_memsets():
                for s in range(2):
                    for g in range(CT):
                        nc.gpsimd.memset(L1[s][g][:], 0.0)
                for s in range(T):
                    for p in range(6):
                        nc.gpsimd.memset(L2[s][p][:], 0.0)

            def emit_ones():
                for t in range(T):
                    nc.sync.dma_start(so4[t][:, 3 * N:4 * N], ones8[:])

            # j-major views of sx: free = jb*2048 + jp*1024 + i*256 + t*64 + n
            sxc = [sx[ct].rearrange("c (jb jp i m) -> c jb jp i m",
                                    jb=2, jp=2, i=4, m=256) for ct in range(CT)]
            sxm = [sx[ct].rearrange("c (jb jp i t n) -> c jb jp i t n",
                                    jb=2, jp=2, i=4, t=4, n=64) for ct in range(CT)]

            # ---- x-LIF (per ct, t-sequential): U>=2 spikes {0,2} fp8,
            #      scattered into the j-major sx tile ----
            def emit_xlif_all(xl):
                # t-major emission; STT is DVE-only on HW, so the recurrence
                # uses a pre-halved state: m = (U<2)*0.5 (DVE TS, 4x), then
                # xv = m*U (TT), then U' = xv + x (TT). Chains alternate
                # engines per ct to pace with the t-major x DMA arrivals.
                ch = [nc.gpsimd, nc.vector, nc.gpsimd]
                for t in range(T):
                    for ct in range(CT):
                        if t == 0:
                            U = xt[0][ct][:]
                        else:
                            Ut = xl.tile([128, N], BF16, tag="xu",
                                         name=f"xu{ct}{t}")
                            ch[ct].tensor_tensor(
                                Ut[:], xv[ct][:], xt[t][ct][:], ALU.add)
                            U = Ut[:]
                        nc.vector.tensor_scalar(
                            sxm[ct][:, :, :, :, t, :], U, 2.0, 2.0,
                            ALU.is_ge, ALU.mult)
                        if t < T - 1:
                            m = xl.tile([128, N], BF16, tag="xm",
                                        name=f"xm{ct}{t}")
                            nc.vector.tensor_scalar(
                                m[:], U, 2.0, 0.5, ALU.is_lt, ALU.mult)
                            ch[ct].tensor_tensor(
                                xv[ct][:], m[:], U, ALU.mult)

            with tc.tile_pool(name="cw", bufs=4) as cw, \
                 tc.tile_pool(name="xl", bufs=3) as xl, \
                 tc.tile_pool(name="tl", bufs=3) as tl, \
                 tc.tile_pool(name="pmp", bufs=2, space="PSUM") as pmp:

                def emit_bn1(mt, pc):
                    dst = y1c[mt] if mt < 3 else y2c[mt - 3]
                    nc.scalar.activation(dst[:], pc[:], ACTF.Identity,
                                         bias=cst[:, 6 + mt:7 + mt],
                                         scale=cst[:, mt:mt + 1])

                def emit_conv_mt(wt, ct, pc, first, last):
                    # 8 fp8 DoubleRow matmuls: (jp, i); ktile pair = jb
                    wv = wt.rearrange("c (jp i jb o) -> c jp i jb o",
                                      jp=2, i=4, jb=2, o=128)
                    for jp in range(2):
                        for i in range(4):
                            nc.tensor.matmul(
                                pc[:], wv[:, jp, i], sxc[ct][:, :, jp, i],
                                start=(first and jp == 0 and i == 0),
                                stop=(last and jp == 1 and i == 3),
                                perf_mode=DR, skip_group_check=True)

                def emit_l1(t, p):
                    g, jj = p // 2, p % 2
                    Ls = L1[t % 2][g]
                    r0 = 64 * jj
                    eng = nc.gpsimd
                    eng.tensor_copy(
                        Ls[r0:r0 + 32, 0:64], y1c[g][r0:r0 + 32,
                                                     t * 64:(t + 1) * 64])
                    eng.tensor_copy(
                        Ls[r0 + 32:r0 + 64, 64:128], y1c[g][r0 + 32:r0 + 64,
                                                            t * 64:(t + 1) * 64])

                def emit_ltrans_pair(t, p):
                        g, jj = p // 2, p % 2
                        Ls = L2[t][p]
                        r0 = 64 * jj
                        c0 = t * 64
                        # hA [32 d, 64 p] -> L2[0:64, 0:32]; hB -> L2[64:128, 32:64]
                        nc.vector.transpose(
                            Ls[0:32, 0:32], y2c[g][r0:r0 + 32, c0:c0 + 32])
                        nc.vector.transpose(
                            Ls[32:64, 0:32], y2c[g][r0:r0 + 32, c0 + 32:c0 + 64])
                        nc.vector.transpose(
                            Ls[64:96, 32:64], y2c[g][r0 + 32:r0 + 64, c0:c0 + 32])
                        nc.vector.transpose(
                            Ls[96:128, 32:64], y2c[g][r0 + 32:r0 + 64,
                                                      c0 + 32:c0 + 64])

                def emit_attn(t, g):
                    # per-pair [1024] tiles; thresholds folded into BN1 ->
                    # uniform theta = 1.0
                    Ua = tl.tile([128, 2 * N], BF16, tag="Ua",
                                 name=f"Ua{t}{g}")
                    for jj in range(2):
                        rA = 64 * jj
                        pm = pmp.tile([128, N], F32, tag="pm",
                                      name=f"pm{t}_{g}{jj}")
                        for nh in range(2):
                            cs = slice(jj * N + nh * 512,
                                       jj * N + nh * 512 + 512)
                            if t > 0:
                                nc.tensor.matmul(
                                    pm[:, nh * 512:(nh + 1) * 512], id05[:],
                                    Gat[g][:, cs],
                                    start=True, stop=False,
                                    skip_group_check=True)
                            nc.tensor.matmul(
                                pm[:, nh * 512:(nh + 1) * 512],
                                L1[t % 2][g][rA:rA + 64, :],
                                sxm[g][rA:rA + 64, nh, :, :, t, :],
                                start=(t == 0), stop=True,
                                skip_group_check=True)
                        nc.scalar.copy(Ua[:, jj * N:(jj + 1) * N], pm[:])
                    nc.vector.tensor_scalar(
                        saT[t][g][:], Ua[:], 1.0, 2.0, ALU.is_ge, ALU.mult)
                    if t < T - 1:
                        m = tl.tile([128, 2 * N], BF16, tag="am",
                                    name=f"am{t}{g}")
                        nc.vector.tensor_scalar(
                            m[:], Ua[:], 1.0, 0.5, ALU.is_lt, ALU.mult)
                        nc.gpsimd.tensor_tensor(
                            Gat[g][:], m[:], Ua[:], ALU.mult)

                pools = {}

                def emit_mm2_outlif(t, g):
                        Uo = tl.tile([128, N], BF16, tag="Uo", bufs=4, name=f"Uo{t}{g}")
                        for nh in range(2):
                            po = pools["pop"].tile([128, 512], F32, tag="po",
                                                   name=f"po{t}{g}{nh}")
                            for jj in range(2):
                                p = 2 * g + jj
                                nc.tensor.matmul(
                                    po[64 * jj:64 * jj + 64, :],
                                    L2[t][p][:, 0:64],
                                    saT[t][g][:, jj * N + nh * 512:
                                              jj * N + nh * 512 + 512],
                                    start=True, stop=True,
                                    tile_position=(0, 64 * jj))
                            if t == 0:
                                nc.scalar.copy(Uo[:, nh * 512:(nh + 1) * 512],
                                               po[:])
                            else:
                                nc.vector.tensor_tensor(
                                    Uo[:, nh * 512:(nh + 1) * 512],
                                    Got[g][:, nh * 512:(nh + 1) * 512],
                                    po[:], ALU.add)
                        nc.gpsimd.tensor_scalar(
                            so4[t][:, g * N:(g + 1) * N], Uo[:],
                            1.0, 2.0, ALU.is_ge, ALU.mult)
                        if t < T - 1:
                            m = tl.tile([128, N], BF16, tag="om", bufs=4,
                                        name=f"om{t}{g}")
                            nc.vector.tensor_scalar(
                                m[:], Uo[:], 1.0, 0.5, ALU.is_lt, ALU.mult)
                            nc.gpsimd.tensor_tensor(Got[g][:], m[:], Uo[:],
                                                    ALU.mult)

                sov4 = [so4[t].rearrange("c (g n) -> c g n", g=4, n=N)
                        for t in range(T)]
                wpv = wpj.rearrange("c (mt dr kt o) -> c mt dr kt o",
                                   mt=3, dr=2, kt=2, o=128)

                def emit_proj_epi(t, mt):
                        # residual x rides the proj group via diag(1/A2c);
                        # epilogue is a pure ACT scale-copy
                        of = tl.tile([128, N], BF16, tag="of", name=f"of{t}{mt}")
                        for nh in range(2):
                            pj = pools["pjp"].tile([128, 512], F32, tag="pj",
                                                   name=f"pj{t}{mt}{nh}")
                            for dr in range(2):
                                nc.tensor.matmul(
                                    pj[:], wpv[:, mt, dr],
                                    sov4[t][:, 2 * dr:2 * dr + 2,
                                            nh * 512:(nh + 1) * 512],
                                    start=(dr == 0), stop=False,
                                    perf_mode=DR, skip_group_check=True)
                            nc.tensor.matmul(
                                pj[:], drt[:, mt * 128:(mt + 1) * 128],
                                xt[t][mt][:, nh * 512:(nh + 1) * 512],
                                start=False, stop=True, skip_group_check=True)
                            nc.scalar.activation(
                                of[:, nh * 512:(nh + 1) * 512], pj[:],
                                ACTF.Copy, bias=0.0,
                                scale=cst[:, 21 + mt:22 + mt])
                        nc.sync.dma_start(y_out[t, mt], of[:])

                # ================= schedule =================
                with tc.tile_pool(name="cp1", bufs=1, space="PSUM") as cp1:
                    pcw1 = [cp1.tile([128, T * NP], F32, tag=f"pcw1_{m}",
                                     name=f"pcw1_{m}") for m in range(3)]
                    # t-major x DMAs so all three chains start early
                    for t in range(T):
                        for ct in range(CT):
                            nc.sync.dma_start(xt[t][ct][:], x_in[t, ct])
                    emit_xlif_all(xl)
                    emit_memsets()
                    for mi, mt in enumerate((0, 1, 2)):
                        for ct in range(CT):
                            wt = cw.tile([128, 2048], FP8, tag="wc",
                                         name=f"w1c{mt}{ct}")
                            nc.sync.dma_start(wt[:], wconv[mt, ct])
                            emit_conv_mt(wt, ct, pcw1[mi],
                                         first=(ct == 0), last=(ct == 2))
                        emit_bn1(mt, pcw1[mi])
                        emit_l1(0, 2 * mi)
                        emit_l1(0, 2 * mi + 1)
                        emit_attn(0, mi)
                    with tc.tile_pool(name="cp2", bufs=1, space="PSUM") as cp2:
                        for g in range(CT):
                            mt = 3 + g
                            pc = cp2.tile([128, T * NP], F32, tag="pc",
                                          name=f"pc{g}")
                            for ct in range(CT):
                                wt = cw.tile([128, 2048], FP8, tag="wc",
                                             name=f"w2c{g}{ct}")
                                nc.sync.dma_start(wt[:], wconv[mt, ct])
                                emit_conv_mt(wt, ct, pc,
                                             first=(ct == 0), last=(ct == 2))
                            emit_bn1(mt, pc)
                            for tt in range(T):
                                emit_ltrans_pair(tt, 2 * g)
                                emit_ltrans_pair(tt, 2 * g + 1)
                emit_ones()
                with tc.tile_pool(name="pop", bufs=2, space="PSUM") as pop_, \
                     tc.tile_pool(name="pjp", bufs=2, space="PSUM") as pjp_:
                    pools["pop"] = pop_
                    pools["pjp"] = pjp_
                    # software pipeline: attn(t+1) ahead of mm2(t); epi lags 2
                    for g in range(CT):
                        emit_l1(1, 2 * g)
                        emit_l1(1, 2 * g + 1)
                        emit_attn(1, g)
                    for g in range(CT):
                        emit_mm2_outlif(0, g)
                    for t in range(2, T):
                        for g in range(CT):
                            emit_l1(t, 2 * g)
                            emit_l1(t, 2 * g + 1)
                            emit_attn(t, g)
                        for g in range(CT):
                            emit_proj_epi(t - 2, g)
                            emit_mm2_outlif(t - 1, g)
                    for g in range(CT):
                        emit_proj_epi(T - 2, g)
                        emit_mm2_outlif(T - 1, g)
                    for g in range(CT):
                        emit_proj_epi(T - 1, g)
    nc.compile()
    return nc


def _host_prep(inputs):
    f32 = np.float32
    w_conv = inputs["w_conv"].astype(f32)
    w_proj = inputs["w_proj"].astype(f32)
    inv1 = inputs["bn1_gamma"] / np.sqrt(inputs["bn1_var"] + EPS)
    B1 = inputs["bn1_beta"] - inv1 * inputs["bn1_mean"]
    inv2 = inputs["bn2_gamma"] / np.sqrt(inputs["bn2_var"] + EPS)
    B2 = inputs["bn2_beta"] - inv2 * inputs["bn2_mean"]
    gam1 = (4.0 * np.sqrt(inputs["fr_x"].reshape(NH) * CH)).astype(f32)
    gam2 = (4.0 * np.sqrt(inputs["fr_attn"].reshape(NH) * NP)).astype(f32)

    # conv output-channel permutation: tile row g*128 + 64*jj + 32*hh + d
    # holds head (4g+2jj+hh): y1 rows from chan h*64+d, y2 from h*64+32+d
    perm = np.empty(2 * C, np.int64)
    d = np.arange(32)
    for g in range(3):
        for jj in range(2):
            for hh in range(2):
                h = 4 * g + 2 * jj + hh
                r = g * 128 + 64 * jj + 32 * hh + d
                perm[r] = h * 64 + d
                perm[384 + r] = h * 64 + 32 + d

    # wconv8 [6, 3, 128, 2048]: free = jp*1024 + i*256 + jb*128 + o
    wc6 = (w_conv[perm] * SC).reshape(6, 128, CT, 128, 4, 4)  # [mt,o,ct,c,i,j]
    w8 = np.zeros((6, CT, 128, 2048), f8np)
    for jp in range(2):
        for i in range(4):
            for jb in range(2):
                j = 2 * jb + jp
                col = jp * 1024 + i * 256 + jb * 128
                w8[:, :, :, col:col + 128] = \
                    wc6[:, :, :, :, i, j].transpose(0, 2, 3, 1).astype(f8np)

    # wproj8 [128, 1536]: free = mt*512 + dr*256 + kt*128 + o
    r = np.arange(128)
    bias_p = (B2 * (2.0 * SP) / inv2).astype(f32)
    wpj8 = np.zeros((128, 1536), f8np)
    for mt in range(3):
        for dridx in range(2):
            for kt in range(2):
                col = mt * 512 + dridx * 256 + kt * 128
                gidx = dridx * 2 + kt
                if gidx < 3:
                    ch = 32 * (4 * gidx + 2 * (r >> 6) + ((r >> 5) & 1)) + (r & 31)
                    wpj8[:, col:col + 128] = \
                        (SP * w_proj[mt * 128:(mt + 1) * 128, ch]).T.astype(f8np)
                else:
                    blk = np.zeros((128, 128), f32)
                    blk[0, :] = bias_p[mt * 128:(mt + 1) * 128]
                    wpj8[:, col:col + 128] = blk.astype(f8np)

    consts = np.zeros((128, 24), f32)
    A1p = inv1[perm] / (2.0 * SC)
    B1p = B1[perm]
    # fold LIF thresholds into BN1: y1 rows /gam1[head], y2 rows /gam2[head]
    r = np.arange(128)
    for g in range(3):
        head = 4 * g + 2 * (r >> 6) + ((r >> 5) & 1)
        A1p[g * 128 + r] /= gam1[head]
        B1p[g * 128 + r] /= gam1[head]
        A1p[384 + g * 128 + r] /= gam2[head]
        B1p[384 + g * 128 + r] /= gam2[head]
    for mt in range(6):
        consts[:, mt] = A1p[mt * 128:(mt + 1) * 128]
        consts[:, 6 + mt] = B1p[mt * 128:(mt + 1) * 128]
    for p in range(6):
        consts[0:64, 12 + p] = gam1[2 * p]
        consts[64:128, 12 + p] = gam1[2 * p + 1]
    for g in range(3):
        consts[:, 18 + g] = np.repeat(gam2[4 * g:4 * g + 4], 32)
        consts[:, 21 + g] = (inv2 / (2.0 * SP))[g * 128:(g + 1) * 128]

    ident05 = np.eye(128, dtype=f32).astype(bf16np)
    A2c = (inv2 / (2.0 * SP)).astype(f32)
    dres_m = np.zeros((128, 3 * 128), f32)
    for mt in range(3):
        dres_m[:, mt * 128:(mt + 1) * 128] = np.diag(
            1.0 / A2c[mt * 128:(mt + 1) * 128])
    dres_m = dres_m.astype(bf16np)

    # column permutation: cInt(n) = (j>>1)*512 + (j&1)*256 + i*64 + hp*8 + wp
    n = np.arange(N)
    hp, rr = n >> 7, n & 127
    ii = rr >> 5
    r2 = rr & 31
    wp = r2 >> 2
    jj = r2 & 3
    cInt = ((jj >> 1) * 512 + (jj & 1) * 256 + ii * 64 + hp * 8 + wp)
    ninv = np.empty(N, np.int64)
    ninv[cInt] = n

    return w8, wpj8, consts, ident05, cInt, ninv, dres_m


def kernel(**inputs):
    inputs = {k: np.asarray(v) for k, v in inputs.items()}
    if "nc" not in _CACHE:
        _CACHE["nc"] = _build_program()
    nc = _CACHE["nc"]

    w8, wpj8, consts, ident05, cInt, ninv, dres_m = _host_prep(inputs)
    x = inputs["x"].astype(np.float32).reshape(T, B, C, N)
    xp = np.ascontiguousarray(x[..., ninv]).astype(bf16np)  # j-major cols

    in_maps = []
    for b in range(8):
        xb = np.ascontiguousarray(xp[:, b].reshape(T, CT, 128, N))
        in_maps.append({"x": xb, "wconv": w8, "wproj": wpj8,
                        "consts": consts, "ident": ident05,
                        "ones8": np.ones((128, N), f8np),
                        "dres": dres_m})

    res = run_bass_kernel_spmd(nc, in_maps, list(range(8)))

    out = np.empty((T, B, C, H, W), dtype=np.float32)
    for b in range(8):
        yb = res.results[b]["y"].astype(np.float32)     # [T, CT, 128, N]
        out[:, b] = yb.reshape(T, C, N)[..., cInt].reshape(T, C, H, W)
    return out


# revision 5
# speedup vs baseline: 1.7354x; 1.0299x over previous
"""Trainium2 Bass kernel v2 for nn_DSSA (spiking self-attention block).

Sharding: data-parallel over B — core b handles batch element b (B=8, 8 cores).

v2 design (validated numerics: ~2.3e-3 norm-rel, dominated by bf16 output):
- x uploaded bf16 with columns permuted j-major (cInt = (j>>1)*512 + (j&1)*256
  + i*64 + hp*8 + wp); y returned bf16 and un-permuted on host.
- conv weights fp8e4m3 (x32), patchify conv = 144 fp8 DoubleRow matmuls
  (K=256 per pass, 0.5 cyc/row) accumulating over (ct, j-pair, i).
- x-LIF spikes {0,2} written straight into the j-major fp8 sx tiles (scattered
  output AP, cost-neutral); MM1 consumes the same fp8 tiles with bf16 lhsT
  (mixed-dtype matmul, HW-verified).
- MM1 block-diagonal: one [64,128] bf16 lhsT per head-pair -> 2 matmuls per
  (t,p) instead of 4; attn state Gat is added in-PSUM by a 0.5*I identity
  matmul prepended to the accumulation group.
- attn evac on ACT, spikes on DVE tensor_scalar (4x mode), resets on Pool STT.
- out-LIF: evac+state-add fused in one DVE STT from PSUM; so spikes fp8 into
  a single [128, 4096] tile (3 g-slices + ones slice) for the proj DoubleRow.
- proj: 4 fp8 DR matmuls per (t,mt) with BN2 bias riding as a k-tile row.
- epilogue: DVE STT (pj*A2c + x) -> bf16 y, 12 output DMAs.
"""

import numpy as np
import ml_dtypes

import concourse.bacc as bacc
import concourse.mybir as mybir
from concourse.tile import TileContext
from concourse.bass_utils import run_bass_kernel_spmd

bf16np = ml_dtypes.bfloat16
f8np = ml_dtypes.float8_e4m3fn
F32 = mybir.dt.float32
BF16 = mybir.dt.bfloat16
FP8 = mybir.dt.float8e4
ALU = mybir.AluOpType
ACTF = mybir.ActivationFunctionType
DR = mybir.MatmulPerfMode.DoubleRow

T, B, C, H, W = 4, 8, 384, 32, 32
NH, CH, P = 12, 32, 4
NP = 64                      # patches
N = H * W                    # 1024
CT = C // 128                # 3 channel tiles
EPS = 1e-5
SC = 32.0                    # conv weight fp8 scale
SP = 16.0                    # proj weight fp8 scale

_CACHE = {}


def _build_program():
    nc = bacc.Bacc("TRN2", target_bir_lowering=False)

    x_in = nc.declare_dram_parameter("x", [T, CT, 128, N], BF16, isOutput=False)
    wconv = nc.declare_dram_parameter("wconv", [6, CT, 128, 2048], FP8, isOutput=False)
    wproj = nc.declare_dram_parameter("wproj", [128, 1536], FP8, isOutput=False)
    consts = nc.declare_dram_parameter("consts", [128, 24], F32, isOutput=False)
    ident = nc.declare_dram_parameter("ident", [128, 128], BF16, isOutput=False)
    ones8 = nc.declare_dram_parameter("ones8", [128, N], FP8, isOutput=False)
    dres = nc.declare_dram_parameter("dres", [128, 3 * 128], BF16, isOutput=False)
    y_out = nc.declare_dram_parameter("y", [T, CT, 128, N], BF16, isOutput=True)

    # consts cols: 0-5 A1p(mt), 6-11 B1p(mt), 12-17 gam1(pair), 18-20 gam2(g),
    # 21-23 A2c(mt)
    with TileContext(nc) as tc:
        with tc.tile_pool(name="sb", bufs=1) as sb:
            cst = sb.tile([128, 24], F32, tag="cst", name="cst")
            nc.sync.dma_start(cst[:], consts[:])
            wpj = sb.tile([128, 1536], FP8, tag="wpj", name="wpj")
            nc.sync.dma_start(wpj[:], wproj[:])
            id05 = sb.tile([128, 128], BF16, tag="id05", name="id05")
            nc.sync.dma_start(id05[:], ident[:])
            drt = sb.tile([128, 3 * 128], BF16, tag="drt", name="drt")
            nc.sync.dma_start(drt[:], dres[:])

            # persistent tiles
            xt = [[sb.tile([128, N], BF16, tag=f"x{t}{ct}", name=f"x{t}{ct}")
                   for ct in range(CT)] for t in range(T)]
            xv = [sb.tile([128, N], BF16, tag=f"xv{ct}", name=f"xv{ct}")
                  for ct in range(CT)]
            sx = [sb.tile([128, T * N], FP8, tag=f"sx{ct}", name=f"sx{ct}")
                  for ct in range(CT)]
            Gat = [sb.tile([128, 2 * N], BF16, tag=f"Gat{g}", name=f"Gat{g}")
                   for g in range(CT)]
            Got = [sb.tile([128, N], BF16, tag=f"Got{g}", name=f"Got{g}")
                   for g in range(CT)]
            y1c = [sb.tile([128, T * NP], BF16, tag=f"y1c{g}", name=f"y1c{g}") for g in range(CT)]
            y2c = [sb.tile([128, T * NP], BF16, tag=f"y2c{g}", name=f"y2c{g}") for g in range(CT)]
            so4 = [sb.tile([128, 4 * N], FP8, tag=f"so4_{t}", name=f"so4_{t}")
                   for t in range(T)]
            saT = [[sb.tile([128, 2 * N], BF16, tag=f"sa{t}_{g}", name=f"sa{t}_{g}")
                    for g in range(CT)] for t in range(T)]
            # MM1 block-diag lhsT (ping-pong by t parity), MM2 lhsT ditto
            L1 = [[sb.tile([128, 128], BF16, tag=f"L1_{s}{g}", name=f"L1_{s}{g}")
                   for g in range(CT)] for s in range(2)]
            L2 = [[sb.tile([128, 64], BF16, tag=f"L2_{s}{p}", name=f"L2_{s}{p}")
                   for p in range(6)] for s in range(T)]
            def emit_memsets():
                for s in range(2):
                    for g in range(CT):
                        nc.gpsimd.memset(L1[s][g][:], 0.0)
                for s in range(T):
                    for p in range(6):
                        nc.gpsimd.memset(L2[s][p][:], 0.0)

            def emit_ones():
                for t in range(T):
                    nc.sync.dma_start(so4[t][:, 3 * N:4 * N], ones8[:])

            # j-major views of sx: free = jb*2048 + jp*1024 + i*256 + t*64 + n
            sxc = [sx[ct].rearrange("c (jb jp i m) -> c jb jp i m",
                                    jb=2, jp=2, i=4, m=256) for ct in range(CT)]
            sxm = [sx[ct].rearrange("c (jb jp i t n) -> c jb jp i t n",
                                    jb=2, jp=2, i=4, t=4, n=64) for ct in range(CT)]

            # ---- x-LIF (per ct, t-sequential): U>=2 spikes {0,2} fp8,
            #      scattered into the j-major sx tile ----
            def emit_xlif_all(xl):
                # t-major emission; STT is DVE-only on HW, so the recurrence
                # uses a pre-halved state: m = (U<2)*0.5 (DVE TS, 4x), then
                # xv = m*U (TT), then U' = xv + x (TT). Chains alternate
                # engines per ct to pace with the t-major x DMA arrivals.
                ch = [nc.gpsimd, nc.vector, nc.gpsimd]
                for t in range(T):
                    for ct in range(CT):
                        if t == 0:
                            U = xt[0][ct][:]
                        else:
                            Ut = xl.tile([128, N], BF16, tag="xu",
                                         name=f"xu{ct}{t}")
                            ch[ct].tensor_tensor(
                                Ut[:], xv[ct][:], xt[t][ct][:], ALU.add)
                            U = Ut[:]
                        nc.vector.tensor_scalar(
                            sxm[ct][:, :, :, :, t, :], U, 2.0, 2.0,
                            ALU.is_ge, ALU.mult)
                        if t < T - 1:
                            m = xl.tile([128, N], BF16, tag="xm",
                                        name=f"xm{ct}{t}")
                            nc.vector.tensor_scalar(
                                m[:], U, 2.0, 0.5, ALU.is_lt, ALU.mult)
                            ch[ct].tensor_tensor(
                                xv[ct][:], m[:], U, ALU.mult)

            with tc.tile_pool(name="cw", bufs=4) as cw, \
                 tc.tile_pool(name="xl", bufs=3) as xl, \
                 tc.tile_pool(name="tl", bufs=3) as tl, \
                 tc.tile_pool(name="pmp", bufs=2, space="PSUM") as pmp:

                def emit_bn1(mt, pc):
                    dst = y1c[mt] if mt < 3 else y2c[mt - 3]
                    nc.scalar.activation(dst[:], pc[:], ACTF.Identity,
                                         bias=cst[:, 6 + mt:7 + mt],
                                         scale=cst[:, mt:mt + 1])

                def emit_conv_mt(wt, ct, pc, first, last):
                    # 8 fp8 DoubleRow matmuls: (jp, i); ktile pair = jb
                    wv = wt.rearrange("c (jp i jb o) -> c jp i jb o",
                                      jp=2, i=4, jb=2, o=128)
                    for jp in range(2):
                        for i in range(4):
                            nc.tensor.matmul(
                                pc[:], wv[:, jp, i], sxc[ct][:, :, jp, i],
                                start=(first and jp == 0 and i == 0),
                                stop=(last and jp == 1 and i == 3),
                                perf_mode=DR, skip_group_check=True)

                def emit_l1(t, p):
                    g, jj = p // 2, p % 2
                    Ls = L1[t % 2][g]
                    r0 = 64 * jj
                    eng = nc.gpsimd
                    eng.tensor_copy(
                        Ls[r0:r0 + 32, 0:64], y1c[g][r0:r0 + 32,
                                                     t * 64:(t + 1) * 64])
                    eng.tensor_copy(
                        Ls[r0 + 32:r0 + 64, 64:128], y1c[g][r0 + 32:r0 + 64,
                                                            t * 64:(t + 1) * 64])

                def emit_ltrans_pair(t, p):
                        g, jj = p // 2, p % 2
                        Ls = L2[t][p]
                        r0 = 64 * jj
                        c0 = t * 64
                        # hA [32 d, 64 p] -> L2[0:64, 0:32]; hB -> L2[64:128, 32:64]
                        nc.vector.transpose(
                            Ls[0:32, 0:32], y2c[g][r0:r0 + 32, c0:c0 + 32])
                        nc.vector.transpose(
                            Ls[32:64, 0:32], y2c[g][r0:r0 + 32, c0 + 32:c0 + 64])
                        nc.vector.transpose(
                            Ls[64:96, 32:64], y2c[g][r0 + 32:r0 + 64, c0:c0 + 32])
                        nc.vector.transpose(
                            Ls[96:128, 32:64], y2c[g][r0 + 32:r0 + 64,
                                                      c0 + 32:c0 + 64])

                def emit_attn(t, g):
                    # per-pair [1024] tiles; thresholds folded into BN1 ->
                    # uniform theta = 1.0
                    Ua = tl.tile([128, 2 * N], BF16, tag="Ua",
                                 name=f"Ua{t}{g}")
                    for jj in range(2):
                        rA = 64 * jj
                        pm = pmp.tile([128, N], F32, tag="pm",
                                      name=f"pm{t}_{g}{jj}")
                        for nh in range(2):
                            cs = slice(jj * N + nh * 512,
                                       jj * N + nh * 512 + 512)
                            if t > 0:
                                nc.tensor.matmul(
                                    pm[:, nh * 512:(nh + 1) * 512], id05[:],
                                    Gat[g][:, cs],
                                    start=True, stop=False,
                                    skip_group_check=True)
                            nc.tensor.matmul(
                                pm[:, nh * 512:(nh + 1) * 512],
                                L1[t % 2][g][rA:rA + 64, :],
                                sxm[g][rA:rA + 64, nh, :, :, t, :],
                                start=(t == 0), stop=True,
                                skip_group_check=True)
                        nc.scalar.copy(Ua[:, jj * N:(jj + 1) * N], pm[:])
                    nc.vector.tensor_scalar(
                        saT[t][g][:], Ua[:], 1.0, 2.0, ALU.is_ge, ALU.mult)
                    if t < T - 1:
                        m = tl.tile([128, 2 * N], BF16, tag="am",
                                    name=f"am{t}{g}")
                        nc.vector.tensor_scalar(
                            m[:], Ua[:], 1.0, 0.5, ALU.is_lt, ALU.mult)
                        nc.gpsimd.tensor_tensor(
                            Gat[g][:], m[:], Ua[:], ALU.mult)

                pools = {}

                def emit_mm2_outlif(t, g):
                        Uo = tl.tile([128, N], BF16, tag="Uo", bufs=4, name=f"Uo{t}{g}")
                        for nh in range(2):
                            po = pools["pop"].tile([128, 512], F32, tag="po",
                                                   name=f"po{t}{g}{nh}")
                            for jj in range(2):
                                p = 2 * g + jj
                                nc.tensor.matmul(
                                    po[64 * jj:64 * jj + 64, :],
                                    L2[t][p][:, 0:64],
                                    saT[t][g][:, jj * N + nh * 512:
                                              jj * N + nh * 512 + 512],
                                    start=True, stop=True,
                                    tile_position=(0, 64 * jj))
                            if t == 0:
                                nc.scalar.copy(Uo[:, nh * 512:(nh + 1) * 512],
                                               po[:])
                            else:
                                nc.vector.tensor_tensor(
                                    Uo[:, nh * 512:(nh + 1) * 512],
                                    Got[g][:, nh * 512:(nh + 1) * 512],
                                    po[:], ALU.add)
                        seng = nc.vector if t == T - 1 else nc.gpsimd
                        seng.tensor_scalar(
                            so4[t][:, g * N:(g + 1) * N], Uo[:],
                            1.0, 2.0, ALU.is_ge, ALU.mult)
                        if t < T - 1:
                            m = tl.tile([128, N], BF16, tag="om", bufs=4,
                                        name=f"om{t}{g}")
                            nc.vector.tensor_scalar(
                                m[:], Uo[:], 1.0, 0.5, ALU.is_lt, ALU.mult)
                            nc.gpsimd.tensor_tensor(Got[g][:], m[:], Uo[:],
                                                    ALU.mult)

                sov4 = [so4[t].rearrange("c (g n) -> c g n", g=4, n=N)
                        for t in range(T)]
                wpv = wpj.rearrange("c (mt dr kt o) -> c mt dr kt o",
                                   mt=3, dr=2, kt=2, o=128)

                def emit_proj_epi(t, mt):
                        # residual x rides the proj group via diag(1/A2c);
                        # epilogue is a pure ACT scale-copy
                        of = tl.tile([128, N], BF16, tag="of", name=f"of{t}{mt}")
                        for nh in range(2):
                            pj = pools["pjp"].tile([128, 512], F32, tag="pj",
                                                   name=f"pj{t}{mt}{nh}")
                            for dr in range(2):
                                nc.tensor.matmul(
                                    pj[:], wpv[:, mt, dr],
                                    sov4[t][:, 2 * dr:2 * dr + 2,
                                            nh * 512:(nh + 1) * 512],
                                    start=(dr == 0), stop=False,
                                    perf_mode=DR, skip_group_check=True)
                            nc.tensor.matmul(
                                pj[:], drt[:, mt * 128:(mt + 1) * 128],
                                xt[t][mt][:, nh * 512:(nh + 1) * 512],
                                start=False, stop=True, skip_group_check=True)
                            nc.scalar.activation(
                                of[:, nh * 512:(nh + 1) * 512], pj[:],
                                ACTF.Copy, bias=0.0,
                                scale=cst[:, 21 + mt:22 + mt])
                            nc.sync.dma_start(
                                y_out[t, mt, :, nh * 512:(nh + 1) * 512],
                                of[:, nh * 512:(nh + 1) * 512])

                # ================= schedule =================
                with tc.tile_pool(name="cp1", bufs=1, space="PSUM") as cp1:
                    pcw1 = [cp1.tile([128, T * NP], F32, tag=f"pcw1_{m}",
                                     name=f"pcw1_{m}") for m in range(3)]
                    # t-major x DMAs so all three chains start early
                    for t in range(T):
                        for ct in range(CT):
                            nc.sync.dma_start(xt[t][ct][:], x_in[t, ct])
                    emit_xlif_all(xl)
                    emit_memsets()
                    for mi, mt in enumerate((0, 1, 2)):
                        for ct in range(CT):
                            wt = cw.tile([128, 2048], FP8, tag="wc",
                                         name=f"w1c{mt}{ct}")
                            nc.sync.dma_start(wt[:], wconv[mt, ct])
                            emit_conv_mt(wt, ct, pcw1[mi],
                                         first=(ct == 0), last=(ct == 2))
                        emit_bn1(mt, pcw1[mi])
                        emit_l1(0, 2 * mi)
                        emit_l1(0, 2 * mi + 1)
                        emit_attn(0, mi)
                    with tc.tile_pool(name="cp2", bufs=1, space="PSUM") as cp2:
                        for g in range(CT):
                            mt = 3 + g
                            pc = cp2.tile([128, T * NP], F32, tag="pc",
                                          name=f"pc{g}")
                            for ct in range(CT):
                                wt = cw.tile([128, 2048], FP8, tag="wc",
                                             name=f"w2c{g}{ct}")
                                nc.sync.dma_start(wt[:], wconv[mt, ct])
                                emit_conv_mt(wt, ct, pc,
                                             first=(ct == 0), last=(ct == 2))
                            emit_bn1(mt, pc)
                            for tt in range(T):
                                emit_ltrans_pair(tt, 2 * g)
                                emit_ltrans_pair(tt, 2 * g + 1)
                emit_ones()
                with tc.tile_pool(name="pop", bufs=2, space="PSUM") as pop_, \
                     tc.tile_pool(name="pjp", bufs=2, space="PSUM") as pjp_:
                    pools["pop"] = pop_
                    pools["pjp"] = pjp_
                    # software pipeline: attn(t+1) ahead of mm2(t); epi lags 2
                    for g in range(CT):
                        emit_l1(1, 2 * g)
                        emit_l1(1, 2 * g + 1)
                        emit_attn(1, g)
                        emit_mm2_outlif(0, g)
                    for t in range(2, T):
                        for g in range(CT):
                            emit_l1(t, 2 * g)
                            emit_l1(t, 2 * g + 1)
                            emit_attn(t, g)
                            emit_mm2_outlif(t - 1, g)
                            emit_proj_epi(t - 2, g)
                    for g in range(CT):
                        emit_mm2_outlif(T - 1, g)
                        emit_proj_epi(T - 2, g)
                    for g in range(CT):
                        emit_proj_epi(T - 1, g)
    nc.compile()
    return nc


def _host_prep(inputs):
    f32 = np.float32
    w_conv = inputs["w_conv"].astype(f32)
    w_proj = inputs["w_proj"].astype(f32)
    inv1 = inputs["bn1_gamma"] / np.sqrt(inputs["bn1_var"] + EPS)
    B1 = inputs["bn1_beta"] - inv1 * inputs["bn1_mean"]
    inv2 = inputs["bn2_gamma"] / np.sqrt(inputs["bn2_var"] + EPS)
    B2 = inputs["bn2_beta"] - inv2 * inputs["bn2_mean"]
    gam1 = (4.0 * np.sqrt(inputs["fr_x"].reshape(NH) * CH)).astype(f32)
    gam2 = (4.0 * np.sqrt(inputs["fr_attn"].reshape(NH) * NP)).astype(f32)

    # conv output-channel permutation: tile row g*128 + 64*jj + 32*hh + d
    # holds head (4g+2jj+hh): y1 rows from chan h*64+d, y2 from h*64+32+d
    perm = np.empty(2 * C, np.int64)
    d = np.arange(32)
    for g in range(3):
        for jj in range(2):
            for hh in range(2):
                h = 4 * g + 2 * jj + hh
                r = g * 128 + 64 * jj + 32 * hh + d
                perm[r] = h * 64 + d
                perm[384 + r] = h * 64 + 32 + d

    # wconv8 [6, 3, 128, 2048]: free = jp*1024 + i*256 + jb*128 + o
    wc6 = (w_conv[perm] * SC).reshape(6, 128, CT, 128, 4, 4)  # [mt,o,ct,c,i,j]
    w8 = np.zeros((6, CT, 128, 2048), f8np)
    for jp in range(2):
        for i in range(4):
            for jb in range(2):
                j = 2 * jb + jp
                col = jp * 1024 + i * 256 + jb * 128
                w8[:, :, :, col:col + 128] = \
                    wc6[:, :, :, :, i, j].transpose(0, 2, 3, 1).astype(f8np)

    # wproj8 [128, 1536]: free = mt*512 + dr*256 + kt*128 + o
    r = np.arange(128)
    bias_p = (B2 * (2.0 * SP) / inv2).astype(f32)
    wpj8 = np.zeros((128, 1536), f8np)
    for mt in range(3):
        for dridx in range(2):
            for kt in range(2):
                col = mt * 512 + dridx * 256 + kt * 128
                gidx = dridx * 2 + kt
                if gidx < 3:
                    ch = 32 * (4 * gidx + 2 * (r >> 6) + ((r >> 5) & 1)) + (r & 31)
                    wpj8[:, col:col + 128] = \
                        (SP * w_proj[mt * 128:(mt + 1) * 128, ch]).T.astype(f8np)
                else:
                    blk = np.zeros((128, 128), f32)
                    blk[0, :] = bias_p[mt * 128:(mt + 1) * 128]
                    wpj8[:, col:col + 128] = blk.astype(f8np)

    consts = np.zeros((128, 24), f32)
    A1p = inv1[perm] / (2.0 * SC)
    B1p = B1[perm]
    # fold LIF thresholds into BN1: y1 rows /gam1[head], y2 rows /gam2[head]
    r = np.arange(128)
    for g in range(3):
        head = 4 * g + 2 * (r >> 6) + ((r >> 5) & 1)
        A1p[g * 128 + r] /= gam1[head]
        B1p[g * 128 + r] /= gam1[head]
        A1p[384 + g * 128 + r] /= gam2[head]
        B1p[384 + g * 128 + r] /= gam2[head]
    for mt in range(6):
        consts[:, mt] = A1p[mt * 128:(mt + 1) * 128]
        consts[:, 6 + mt] = B1p[mt * 128:(mt + 1) * 128]
    for p in range(6):
        consts[0:64, 12 + p] = gam1[2 * p]
        consts[64:128, 12 + p] = gam1[2 * p + 1]
    for g in range(3):
        consts[:, 18 + g] = np.repeat(gam2[4 * g:4 * g + 4], 32)
        consts[:, 21 + g] = (inv2 / (2.0 * SP))[g * 128:(g + 1) * 128]

    ident05 = np.eye(128, dtype=f32).astype(bf16np)
    A2c = (inv2 / (2.0 * SP)).astype(f32)
    dres_m = np.zeros((128, 3 * 128), f32)
    for mt in range(3):
        dres_m[:, mt * 128:(mt + 1) * 128] = np.diag(
            1.0 / A2c[mt * 128:(mt + 1) * 128])
    dres_m = dres_m.astype(bf16np)

    # column permutation: cInt(n) = (j>>1)*512 + (j&1)*256 + i*64 + hp*8 + wp
    n = np.arange(N)
    hp, rr = n >> 7, n & 127
    ii = rr >> 5
    r2 = rr & 31
    wp = r2 >> 2
    jj = r2 & 3
    cInt = ((jj >> 1) * 512 + (jj & 1) * 256 + ii * 64 + hp * 8 + wp)
    ninv = np.empty(N, np.int64)
    ninv[cInt] = n

    return w8, wpj8, consts, ident05, cInt, ninv, dres_m


def kernel(**inputs):
    inputs = {k: np.asarray(v) for k, v in inputs.items()}
    if "nc" not in _CACHE:
        _CACHE["nc"] = _build_program()
    nc = _CACHE["nc"]

    w8, wpj8, consts, ident05, cInt, ninv, dres_m = _host_prep(inputs)
    x = inputs["x"].astype(np.float32).reshape(T, B, C, N)
    xp = np.ascontiguousarray(x[..., ninv]).astype(bf16np)  # j-major cols

    in_maps = []
    for b in range(8):
        xb = np.ascontiguousarray(xp[:, b].reshape(T, CT, 128, N))
        in_maps.append({"x": xb, "wconv": w8, "wproj": wpj8,
                        "consts": consts, "ident": ident05,
                        "ones8": np.ones((128, N), f8np),
                        "dres": dres_m})

    res = run_bass_kernel_spmd(nc, in_maps, list(range(8)))

    out = np.empty((T, B, C, H, W), dtype=np.float32)
    for b in range(8):
        yb = res.results[b]["y"].astype(np.float32)     # [T, CT, 128, N]
        out[:, b] = yb.reshape(T, C, N)[..., cInt].reshape(T, C, H, W)
    return out


# revision 7
# speedup vs baseline: 1.7681x; 1.0189x over previous
"""Trainium2 Bass kernel v2 for nn_DSSA (spiking self-attention block).

Sharding: data-parallel over B — core b handles batch element b (B=8, 8 cores).

v2 design (validated numerics: ~2.3e-3 norm-rel, dominated by bf16 output):
- x uploaded bf16 with columns permuted j-major (cInt = (j>>1)*512 + (j&1)*256
  + i*64 + hp*8 + wp); y returned bf16 and un-permuted on host.
- conv weights fp8e4m3 (x32), patchify conv = 144 fp8 DoubleRow matmuls
  (K=256 per pass, 0.5 cyc/row) accumulating over (ct, j-pair, i).
- x-LIF spikes {0,2} written straight into the j-major fp8 sx tiles (scattered
  output AP, cost-neutral); MM1 consumes the same fp8 tiles with bf16 lhsT
  (mixed-dtype matmul, HW-verified).
- MM1 block-diagonal: one [64,128] bf16 lhsT per head-pair -> 2 matmuls per
  (t,p) instead of 4; attn state Gat is added in-PSUM by a 0.5*I identity
  matmul prepended to the accumulation group.
- attn evac on ACT, spikes on DVE tensor_scalar (4x mode), resets on Pool STT.
- out-LIF: evac+state-add fused in one DVE STT from PSUM; so spikes fp8 into
  a single [128, 4096] tile (3 g-slices + ones slice) for the proj DoubleRow.
- proj: 4 fp8 DR matmuls per (t,mt) with BN2 bias riding as a k-tile row.
- epilogue: DVE STT (pj*A2c + x) -> bf16 y, 12 output DMAs.
"""

import numpy as np
import ml_dtypes

import concourse.bacc as bacc
import concourse.mybir as mybir
from concourse.tile import TileContext
from concourse.bass_utils import run_bass_kernel_spmd

bf16np = ml_dtypes.bfloat16
f8np = ml_dtypes.float8_e4m3fn
F32 = mybir.dt.float32
BF16 = mybir.dt.bfloat16
FP8 = mybir.dt.float8e4
ALU = mybir.AluOpType
ACTF = mybir.ActivationFunctionType
DR = mybir.MatmulPerfMode.DoubleRow

T, B, C, H, W = 4, 8, 384, 32, 32
NH, CH, P = 12, 32, 4
NP = 64                      # patches
N = H * W                    # 1024
CT = C // 128                # 3 channel tiles
EPS = 1e-5
SC = 32.0                    # conv weight fp8 scale
SP = 16.0                    # proj weight fp8 scale

_CACHE = {}


def _build_program():
    nc = bacc.Bacc("TRN2", target_bir_lowering=False)

    x_in = nc.declare_dram_parameter("x", [T, CT, 128, N], BF16, isOutput=False)
    wconv = nc.declare_dram_parameter("wconv", [6, CT, 128, 2048], FP8, isOutput=False)
    wproj = nc.declare_dram_parameter("wproj", [128, 1536], FP8, isOutput=False)
    consts = nc.declare_dram_parameter("consts", [128, 24], F32, isOutput=False)
    ident = nc.declare_dram_parameter("ident", [128, 128], BF16, isOutput=False)
    ones8 = nc.declare_dram_parameter("ones8", [128, N], FP8, isOutput=False)
    dres = nc.declare_dram_parameter("dres", [128, 3 * 128], BF16, isOutput=False)
    y_out = nc.declare_dram_parameter("y", [T, CT, 128, N], BF16, isOutput=True)

    # consts cols: 0-5 A1p(mt), 6-11 B1p(mt), 12-17 gam1(pair), 18-20 gam2(g),
    # 21-23 A2c(mt)
    with TileContext(nc) as tc:
        with tc.tile_pool(name="sb", bufs=1) as sb:
            cst = sb.tile([128, 24], F32, tag="cst", name="cst")
            nc.sync.dma_start(cst[:], consts[:])
            wpj = sb.tile([128, 1536], FP8, tag="wpj", name="wpj")
            nc.sync.dma_start(wpj[:], wproj[:])
            id05 = sb.tile([128, 128], BF16, tag="id05", name="id05")
            nc.sync.dma_start(id05[:], ident[:])
            drt = sb.tile([128, 3 * 128], BF16, tag="drt", name="drt")
            nc.sync.dma_start(drt[:], dres[:])

            # persistent tiles
            xt = [[sb.tile([128, N], BF16, tag=f"x{t}{ct}", name=f"x{t}{ct}")
                   for ct in range(CT)] for t in range(T)]
            xv = [sb.tile([128, N], BF16, tag=f"xv{ct}", name=f"xv{ct}")
                  for ct in range(CT)]
            sx = [sb.tile([128, T * N], FP8, tag=f"sx{ct}", name=f"sx{ct}")
                  for ct in range(CT)]
            Gat = [sb.tile([128, 2 * N], BF16, tag=f"Gat{g}", name=f"Gat{g}")
                   for g in range(CT)]
            Got = [sb.tile([128, N], BF16, tag=f"Got{g}", name=f"Got{g}")
                   for g in range(CT)]
            y1c = [sb.tile([128, T * NP], BF16, tag=f"y1c{g}", name=f"y1c{g}") for g in range(CT)]
            y2c = [sb.tile([128, T * NP], BF16, tag=f"y2c{g}", name=f"y2c{g}") for g in range(CT)]
            so4 = [sb.tile([128, 4 * N], FP8, tag=f"so4_{t}", name=f"so4_{t}")
                   for t in range(T)]
            saT = [[sb.tile([128, 2 * N], BF16, tag=f"sa{t}_{g}", name=f"sa{t}_{g}")
                    for g in range(CT)] for t in range(T)]
            # MM1 block-diag lhsT (ping-pong by t parity), MM2 lhsT ditto
            L1 = [[sb.tile([128, 128], BF16, tag=f"L1_{s}{g}", name=f"L1_{s}{g}")
                   for g in range(CT)] for s in range(2)]
            L2 = [[sb.tile([128, 64], BF16, tag=f"L2_{s}{p}", name=f"L2_{s}{p}")
                   for p in range(6)] for s in range(T)]
            def emit_memsets():
                for s in range(2):
                    for g in range(CT):
                        nc.gpsimd.memset(L1[s][g][:], 0.0)
                for s in range(T):
                    for p in range(6):
                        nc.gpsimd.memset(L2[s][p][:], 0.0)

            def emit_ones():
                for t in range(T):
                    nc.sync.dma_start(so4[t][:, 3 * N:4 * N], ones8[:])

            # j-major views of sx: free = jb*2048 + jp*1024 + i*256 + t*64 + n
            sxc = [sx[ct].rearrange("c (jb jp i m) -> c jb jp i m",
                                    jb=2, jp=2, i=4, m=256) for ct in range(CT)]
            sxm = [sx[ct].rearrange("c (jb jp i t n) -> c jb jp i t n",
                                    jb=2, jp=2, i=4, t=4, n=64) for ct in range(CT)]

            # ---- x-LIF (per ct, t-sequential): U>=2 spikes {0,2} fp8,
            #      scattered into the j-major sx tile ----
            def emit_xlif_all(xl):
                # t-major emission; STT is DVE-only on HW, so the recurrence
                # uses a pre-halved state: m = (U<2)*0.5 (DVE TS, 4x), then
                # xv = m*U (TT), then U' = xv + x (TT). Chains alternate
                # engines per ct to pace with the t-major x DMA arrivals.
                ch = [nc.gpsimd, nc.vector, nc.gpsimd]
                sp = [nc.vector, nc.vector, nc.vector]
                for t in range(T):
                    for ct in range(CT):
                        if t == 0:
                            U = xt[0][ct][:]
                        else:
                            Ut = xl.tile([128, N], BF16, tag="xu",
                                         name=f"xu{ct}{t}")
                            ch[ct].tensor_tensor(
                                Ut[:], xv[ct][:], xt[t][ct][:], ALU.add)
                            U = Ut[:]
                        sp[ct].tensor_scalar(
                            sxm[ct][:, :, :, :, t, :], U, 2.0, 2.0,
                            ALU.is_ge, ALU.mult)
                        if t < T - 1:
                            m = xl.tile([128, N], BF16, tag="xm",
                                        name=f"xm{ct}{t}")
                            nc.vector.tensor_scalar(
                                m[:], U, 2.0, 0.5, ALU.is_lt, ALU.mult)
                            ch[ct].tensor_tensor(
                                xv[ct][:], m[:], U, ALU.mult)

            with tc.tile_pool(name="cw", bufs=4) as cw, \
                 tc.tile_pool(name="xl", bufs=3) as xl, \
                 tc.tile_pool(name="tl", bufs=3) as tl, \
                 tc.tile_pool(name="pmp", bufs=2, space="PSUM") as pmp:

                def emit_bn1(mt, pc):
                    dst = y1c[mt] if mt < 3 else y2c[mt - 3]
                    nc.scalar.activation(dst[:], pc[:], ACTF.Identity,
                                         bias=cst[:, 6 + mt:7 + mt],
                                         scale=cst[:, mt:mt + 1])

                def emit_conv_mt(wt, ct, pc, first, last):
                    # 8 fp8 DoubleRow matmuls: (jp, i); ktile pair = jb
                    wv = wt.rearrange("c (jp i jb o) -> c jp i jb o",
                                      jp=2, i=4, jb=2, o=128)
                    for jp in range(2):
                        for i in range(4):
                            nc.tensor.matmul(
                                pc[:], wv[:, jp, i], sxc[ct][:, :, jp, i],
                                start=(first and jp == 0 and i == 0),
                                stop=(last and jp == 1 and i == 3),
                                perf_mode=DR, skip_group_check=True)

                def emit_l1(t, p):
                    g, jj = p // 2, p % 2
                    Ls = L1[t % 2][g]
                    r0 = 64 * jj
                    eng = nc.gpsimd
                    eng.tensor_copy(
                        Ls[r0:r0 + 32, 0:64], y1c[g][r0:r0 + 32,
                                                     t * 64:(t + 1) * 64])
                    eng.tensor_copy(
                        Ls[r0 + 32:r0 + 64, 64:128], y1c[g][r0 + 32:r0 + 64,
                                                            t * 64:(t + 1) * 64])

                def emit_ltrans_pair(t, p):
                        g, jj = p // 2, p % 2
                        Ls = L2[t][p]
                        r0 = 64 * jj
                        c0 = t * 64
                        # hA [32 d, 64 p] -> L2[0:64, 0:32]; hB -> L2[64:128, 32:64]
                        nc.vector.transpose(
                            Ls[0:32, 0:32], y2c[g][r0:r0 + 32, c0:c0 + 32])
                        nc.vector.transpose(
                            Ls[32:64, 0:32], y2c[g][r0:r0 + 32, c0 + 32:c0 + 64])
                        nc.vector.transpose(
                            Ls[64:96, 32:64], y2c[g][r0 + 32:r0 + 64, c0:c0 + 32])
                        nc.vector.transpose(
                            Ls[96:128, 32:64], y2c[g][r0 + 32:r0 + 64,
                                                      c0 + 32:c0 + 64])

                def emit_attn(t, g):
                    # per-pair [1024] tiles; thresholds folded into BN1 ->
                    # uniform theta = 1.0
                    Ua = tl.tile([128, 2 * N], BF16, tag="Ua",
                                 name=f"Ua{t}{g}")
                    for jj in range(2):
                        rA = 64 * jj
                        pm = pmp.tile([128, N], F32, tag="pm",
                                      name=f"pm{t}_{g}{jj}")
                        for nh in range(2):
                            nc.tensor.matmul(
                                pm[:, nh * 512:(nh + 1) * 512],
                                L1[t % 2][g][rA:rA + 64, :],
                                sxm[g][rA:rA + 64, nh, :, :, t, :],
                                start=True, stop=(t == 0),
                                skip_group_check=True)
                        if t > 0:
                            for nh in range(2):
                                cs = slice(jj * N + nh * 512,
                                           jj * N + nh * 512 + 512)
                                nc.tensor.matmul(
                                    pm[:, nh * 512:(nh + 1) * 512], id05[:],
                                    Gat[g][:, cs],
                                    start=False, stop=True,
                                    skip_group_check=True)
                        nc.scalar.copy(Ua[:, jj * N:(jj + 1) * N], pm[:])
                    nc.vector.tensor_scalar(
                        saT[t][g][:], Ua[:], 1.0, 2.0, ALU.is_ge, ALU.mult)
                    if t < T - 1:
                        m = tl.tile([128, 2 * N], BF16, tag="am",
                                    name=f"am{t}{g}")
                        nc.vector.tensor_scalar(
                            m[:], Ua[:], 1.0, 0.5, ALU.is_lt, ALU.mult)
                        nc.gpsimd.tensor_tensor(
                            Gat[g][:], m[:], Ua[:], ALU.mult)

                pools = {}

                def emit_mm2_outlif(t, g):
                        Uo = tl.tile([128, N], BF16, tag="Uo", bufs=4, name=f"Uo{t}{g}")
                        pos = []
                        for nh in range(2):
                            po = pools["pop"].tile([128, 512], F32, tag="po",
                                                   name=f"po{t}{g}{nh}")
                            pos.append(po)
                            for jj in range(2):
                                p = 2 * g + jj
                                nc.tensor.matmul(
                                    po[64 * jj:64 * jj + 64, :],
                                    L2[t][p][:, 0:64],
                                    saT[t][g][:, jj * N + nh * 512:
                                              jj * N + nh * 512 + 512],
                                    start=True, stop=True,
                                    tile_position=(0, 64 * jj))
                        for nh in range(2):
                            po = pos[nh]
                            if t == 0:
                                nc.scalar.copy(Uo[:, nh * 512:(nh + 1) * 512],
                                               po[:])
                            else:
                                nc.vector.tensor_tensor(
                                    Uo[:, nh * 512:(nh + 1) * 512],
                                    Got[g][:, nh * 512:(nh + 1) * 512],
                                    po[:], ALU.add)
                        seng = nc.vector if t == T - 1 else nc.gpsimd
                        seng.tensor_scalar(
                            so4[t][:, g * N:(g + 1) * N], Uo[:],
                            1.0, 2.0, ALU.is_ge, ALU.mult)
                        if t < T - 1:
                            m = tl.tile([128, N], BF16, tag="om", bufs=4,
                                        name=f"om{t}{g}")
                            nc.vector.tensor_scalar(
                                m[:], Uo[:], 1.0, 0.5, ALU.is_lt, ALU.mult)
                            nc.gpsimd.tensor_tensor(Got[g][:], m[:], Uo[:],
                                                    ALU.mult)

                sov4 = [so4[t].rearrange("c (g n) -> c g n", g=4, n=N)
                        for t in range(T)]
                wpv = wpj.rearrange("c (mt dr kt o) -> c mt dr kt o",
                                   mt=3, dr=2, kt=2, o=128)

                def emit_proj_epi(t, mt):
                        # residual x rides the proj group via diag(1/A2c);
                        # epilogue is a pure ACT scale-copy
                        of = tl.tile([128, N], BF16, tag="of", name=f"of{t}{mt}")
                        pjs = []
                        for nh in range(2):
                            pj = pools["pjp"].tile([128, 512], F32, tag="pj",
                                                   name=f"pj{t}{mt}{nh}")
                            pjs.append(pj)
                            nc.tensor.matmul(
                                pj[:], drt[:, mt * 128:(mt + 1) * 128],
                                xt[t][mt][:, nh * 512:(nh + 1) * 512],
                                start=True, stop=False, skip_group_check=True)
                            for dr in range(2):
                                nc.tensor.matmul(
                                    pj[:], wpv[:, mt, dr],
                                    sov4[t][:, 2 * dr:2 * dr + 2,
                                            nh * 512:(nh + 1) * 512],
                                    start=False, stop=(dr == 1),
                                    perf_mode=DR, skip_group_check=True)
                        for nh in range(2):
                            nc.scalar.activation(
                                of[:, nh * 512:(nh + 1) * 512], pjs[nh][:],
                                ACTF.Copy, bias=0.0,
                                scale=cst[:, 21 + mt:22 + mt])
                            nc.sync.dma_start(
                                y_out[t, mt, :, nh * 512:(nh + 1) * 512],
                                of[:, nh * 512:(nh + 1) * 512])

                # ================= schedule =================
                with tc.tile_pool(name="cp1", bufs=1, space="PSUM") as cp1:
                    pcw1 = [cp1.tile([128, T * NP], F32, tag=f"pcw1_{m}",
                                     name=f"pcw1_{m}") for m in range(3)]
                    # t-major x DMAs so all three chains start early
                    for t in range(T):
                        for ct in range(CT):
                            nc.sync.dma_start(xt[t][ct][:], x_in[t, ct])
                    emit_xlif_all(xl)
                    emit_memsets()
                    for mi, mt in enumerate((0, 1, 2)):
                        for ct in range(CT):
                            wt = cw.tile([128, 2048], FP8, tag="wc",
                                         name=f"w1c{mt}{ct}")
                            nc.sync.dma_start(wt[:], wconv[mt, ct])
                            emit_conv_mt(wt, ct, pcw1[mi],
                                         first=(ct == 0), last=(ct == 2))
                        emit_bn1(mt, pcw1[mi])
                        emit_l1(0, 2 * mi)
                        emit_l1(0, 2 * mi + 1)
                        emit_attn(0, mi)
                    with tc.tile_pool(name="cp2", bufs=1, space="PSUM") as cp2:
                        for g in range(CT):
                            mt = 3 + g
                            pc = cp2.tile([128, T * NP], F32, tag="pc",
                                          name=f"pc{g}")
                            for ct in range(CT):
                                wt = cw.tile([128, 2048], FP8, tag="wc",
                                             name=f"w2c{g}{ct}")
                                nc.sync.dma_start(wt[:], wconv[mt, ct])
                                emit_conv_mt(wt, ct, pc,
                                             first=(ct == 0), last=(ct == 2))
                            emit_bn1(mt, pc)
                            for tt in range(T):
                                emit_ltrans_pair(tt, 2 * g)
                                emit_ltrans_pair(tt, 2 * g + 1)
                emit_ones()
                with tc.tile_pool(name="pop", bufs=2, space="PSUM") as pop_, \
                     tc.tile_pool(name="pjp", bufs=2, space="PSUM") as pjp_:
                    pools["pop"] = pop_
                    pools["pjp"] = pjp_
                    # software pipeline: attn(t+1) ahead of mm2(t); epi lags 2
                    for g in range(CT):
                        emit_l1(1, 2 * g)
                        emit_l1(1, 2 * g + 1)
                        emit_attn(1, g)
                        emit_mm2_outlif(0, g)
                    for t in range(2, T):
                        for g in range(CT):
                            emit_l1(t, 2 * g)
                            emit_l1(t, 2 * g + 1)
                            emit_attn(t, g)
                            emit_mm2_outlif(t - 1, g)
                            emit_proj_epi(t - 2, g)
                    for g in range(CT):
                        emit_mm2_outlif(T - 1, g)
                        emit_proj_epi(T - 2, g)
                    for g in range(CT):
                        emit_proj_epi(T - 1, g)
    nc.compile()
    return nc


def _host_prep(inputs):
    f32 = np.float32
    w_conv = inputs["w_conv"].astype(f32)
    w_proj = inputs["w_proj"].astype(f32)
    inv1 = inputs["bn1_gamma"] / np.sqrt(inputs["bn1_var"] + EPS)
    B1 = inputs["bn1_beta"] - inv1 * inputs["bn1_mean"]
    inv2 = inputs["bn2_gamma"] / np.sqrt(inputs["bn2_var"] + EPS)
    B2 = inputs["bn2_beta"] - inv2 * inputs["bn2_mean"]
    gam1 = (4.0 * np.sqrt(inputs["fr_x"].reshape(NH) * CH)).astype(f32)
    gam2 = (4.0 * np.sqrt(inputs["fr_attn"].reshape(NH) * NP)).astype(f32)

    # conv output-channel permutation: tile row g*128 + 64*jj + 32*hh + d
    # holds head (4g+2jj+hh): y1 rows from chan h*64+d, y2 from h*64+32+d
    perm = np.empty(2 * C, np.int64)
    d = np.arange(32)
    for g in range(3):
        for jj in range(2):
            for hh in range(2):
                h = 4 * g + 2 * jj + hh
                r = g * 128 + 64 * jj + 32 * hh + d
                perm[r] = h * 64 + d
                perm[384 + r] = h * 64 + 32 + d

    # wconv8 [6, 3, 128, 2048]: free = jp*1024 + i*256 + jb*128 + o
    wc6 = (w_conv[perm] * SC).reshape(6, 128, CT, 128, 4, 4)  # [mt,o,ct,c,i,j]
    w8 = np.zeros((6, CT, 128, 2048), f8np)
    for jp in range(2):
        for i in range(4):
            for jb in range(2):
                j = 2 * jb + jp
                col = jp * 1024 + i * 256 + jb * 128
                w8[:, :, :, col:col + 128] = \
                    wc6[:, :, :, :, i, j].transpose(0, 2, 3, 1).astype(f8np)

    # wproj8 [128, 1536]: free = mt*512 + dr*256 + kt*128 + o
    r = np.arange(128)
    bias_p = (B2 * (2.0 * SP) / inv2).astype(f32)
    wpj8 = np.zeros((128, 1536), f8np)
    for mt in range(3):
        for dridx in range(2):
            for kt in range(2):
                col = mt * 512 + dridx * 256 + kt * 128
                gidx = dridx * 2 + kt
                if gidx < 3:
                    ch = 32 * (4 * gidx + 2 * (r >> 6) + ((r >> 5) & 1)) + (r & 31)
                    wpj8[:, col:col + 128] = \
                        (SP * w_proj[mt * 128:(mt + 1) * 128, ch]).T.astype(f8np)
                else:
                    blk = np.zeros((128, 128), f32)
                    blk[0, :] = bias_p[mt * 128:(mt + 1) * 128]
                    wpj8[:, col:col + 128] = blk.astype(f8np)

    consts = np.zeros((128, 24), f32)
    A1p = inv1[perm] / (2.0 * SC)
    B1p = B1[perm]
    # fold LIF thresholds into BN1: y1 rows /gam1[head], y2 rows /gam2[head]
    r = np.arange(128)
    for g in range(3):
        head = 4 * g + 2 * (r >> 6) + ((r >> 5) & 1)
        A1p[g * 128 + r] /= gam1[head]
        B1p[g * 128 + r] /= gam1[head]
        A1p[384 + g * 128 + r] /= gam2[head]
        B1p[384 + g * 128 + r] /= gam2[head]
    for mt in range(6):
        consts[:, mt] = A1p[mt * 128:(mt + 1) * 128]
        consts[:, 6 + mt] = B1p[mt * 128:(mt + 1) * 128]
    for p in range(6):
        consts[0:64, 12 + p] = gam1[2 * p]
        consts[64:128, 12 + p] = gam1[2 * p + 1]
    for g in range(3):
        consts[:, 18 + g] = np.repeat(gam2[4 * g:4 * g + 4], 32)
        consts[:, 21 + g] = (inv2 / (2.0 * SP))[g * 128:(g + 1) * 128]

    ident05 = np.eye(128, dtype=f32).astype(bf16np)
    A2c = (inv2 / (2.0 * SP)).astype(f32)
    dres_m = np.zeros((128, 3 * 128), f32)
    for mt in range(3):
        dres_m[:, mt * 128:(mt + 1) * 128] = np.diag(
            1.0 / A2c[mt * 128:(mt + 1) * 128])
    dres_m = dres_m.astype(bf16np)

    # column permutation: cInt(n) = (j>>1)*512 + (j&1)*256 + i*64 + hp*8 + wp
    n = np.arange(N)
    hp, rr = n >> 7, n & 127
    ii = rr >> 5
    r2 = rr & 31
    wp = r2 >> 2
    jj = r2 & 3
    cInt = ((jj >> 1) * 512 + (jj & 1) * 256 + ii * 64 + hp * 8 + wp)
    ninv = np.empty(N, np.int64)
    ninv[cInt] = n

    return w8, wpj8, consts, ident05, cInt, ninv, dres_m


def kernel(**inputs):
    inputs = {k: np.asarray(v) for k, v in inputs.items()}
    if "nc" not in _CACHE:
        _CACHE["nc"] = _build_program()
    nc = _CACHE["nc"]

    w8, wpj8, consts, ident05, cInt, ninv, dres_m = _host_prep(inputs)
    x = inputs["x"].astype(np.float32).reshape(T, B, C, N)
    xp = np.ascontiguousarray(x[..., ninv]).astype(bf16np)  # j-major cols

    in_maps = []
    for b in range(8):
        xb = np.ascontiguousarray(xp[:, b].reshape(T, CT, 128, N))
        in_maps.append({"x": xb, "wconv": w8, "wproj": wpj8,
                        "consts": consts, "ident": ident05,
                        "ones8": np.ones((128, N), f8np),
                        "dres": dres_m})

    res = run_bass_kernel_spmd(nc, in_maps, list(range(8)))

    out = np.empty((T, B, C, H, W), dtype=np.float32)
    for b in range(8):
        yb = res.results[b]["y"].astype(np.float32)     # [T, CT, 128, N]
        out[:, b] = yb.reshape(T, C, N)[..., cInt].reshape(T, C, H, W)
    return out
